# revision 1
# baseline (speedup 1.0000x reference)
"""GATv2 message passing on 8 Trainium2 NeuronCores (Bass/Tile).

Strategy (edge-parallel by receiver ownership):
  - Nodes are split into 8 contiguous ranges of 6250; core c owns range c and
    all edges whose receiver falls in it (so no cross-core reduction at all).
  - Each core projects the full node table with Ws (and its local slice with
    Wr) on the PE into DRAM scratch tables, then streams its edge shard:
    dma_gather of s-proj/r-proj rows, mish + attention logits, exp (softmax
    without max-subtraction: logits are O(1) so exp is safe in f32),
    and a one-hot matmul scatter into per-window PSUM accumulators.
  - Edges are sorted by (128-node window, sender>=32768) host-side; each
    (window, half) run is padded to a multiple of 128 so the SPMD program
    structure is uniform across cores.  Padding edges gather row 0 and carry
    a sentinel receiver (999) whose one-hot row is all-zero, so they
    contribute nothing to either numerator or denominator.
  - out[n] = segsum(exp(logit)*msg) / segsum(exp(logit)), computed on-device;
    host only reassembles the [50000,128] output from the 8 slices.
"""

import os
import sys

for _p in ("/opt/trn_rl_repo", "/root/.axon_site/_ro/trn_rl_repo"):
    if os.path.isdir(_p) and _p not in sys.path:
        sys.path.insert(0, _p)

import numpy as np

import concourse.bass as bass
import concourse.bacc as bacc
import concourse.tile as tile
from concourse import mybir
from concourse import bass_utils
from concourse.masks import make_identity

F32 = mybir.dt.float32
I16 = mybir.dt.int16

N_NODES = 50000
N_EDGES = 800000
F = 128            # feature dim
H = 8              # heads
D = 16             # head dim
NCORE = 8
NPC = N_NODES // NCORE          # 6250 nodes per core
WIN = 128                       # nodes per scatter window
NWIN = (NPC + WIN - 1) // WIN   # 49 windows per core
SPLIT = 32768                   # int16 gather-index limit -> lo/hi tables
NP_PAD = 50176                  # nodes padded to 98*512 for precompute
NL_PAD = 6656                   # local nodes padded to 13*512
HI_ROWS = NP_PAD - SPLIT        # 17408
CHUNK = 128                     # edges per matmul chunk
GRP = 8                         # chunks per elementwise group
BLK = 32                        # chunks per DMA block (4096 edges)

_prog_cache = {}


def _build_program(lo_ch, hi_ch, nblk, attn_bias):
    """Build the SPMD Bass program for chunk structure (lo_ch, hi_ch)."""
    debug_phase = os.environ.get("GAT_PHASE", "")
    cpw = lo_ch + hi_ch                      # chunks per window
    n_real = NWIN * cpw                      # real chunks in stream
    n_chunks = nblk * BLK

    # chunk -> table ('lo'/'hi'), window, pos-in-window
    def chunk_info(g):
        if g >= n_real:
            return ("lo", None, None)
        w, pos = divmod(g, cpw)
        return ("lo" if pos < lo_ch else "hi", w, pos)

    nc = bacc.Bacc("TRN2", target_bir_lowering=False, debug=False,
                   enable_asserts=False, num_devices=NCORE)

    inp = {}
    def dram_in(name, shape, dt=F32):
        inp[name] = nc.dram_tensor(name, list(shape), dt, kind="ExternalInput").ap()
        return inp[name]

    nodes_pad = dram_in("nodes_pad", (NP_PAD, F))
    nodes_loc = dram_in("nodes_loc", (NL_PAD, F))
    ws_mat = dram_in("ws_mat", (F, F))
    wr_mat = dram_in("wr_mat", (F, F))
    wsb = dram_in("wsb", (1, F))
    wrb = dram_in("wrb", (1, F))
    iota_in = dram_in("iota", (128, 128))
    attn_in = dram_in("attn_rep", (128, 128))
    sidx_in = dram_in("sidx", (nblk, 128, BLK * CHUNK // 16), I16)
    ridx_in = dram_in("ridx", (nblk, 128, BLK * CHUNK // 16), I16)
    rloc_in = dram_in("rloc", (nblk, 128, BLK))
    out_d = nc.dram_tensor("out_d", [NWIN * WIN, F], F32, kind="ExternalOutput").ap()

    tab_lo = nc.dram_tensor("tab_lo", [SPLIT, F], F32, kind="Internal").ap()
    tab_hi = nc.dram_tensor("tab_hi", [HI_ROWS, F], F32, kind="Internal").ap()
    tab_r = nc.dram_tensor("tab_r", [NL_PAD, F], F32, kind="Internal").ap()

    with tile.TileContext(nc) as tc:
        # ---------------- Phase 1: projection precompute ----------------
        with tc.tile_pool(name="pp_const", bufs=1) as cpool, \
             tc.tile_pool(name="pp_sbuf", bufs=3) as spool, \
             tc.tile_pool(name="pp_psum", bufs=2, space="PSUM") as ppool:
            ident = cpool.tile([128, 128], F32)
            make_identity(nc, ident[:])
            ws_t = cpool.tile([F, F], F32)
            wr_t = cpool.tile([F, F], F32)
            wsb_t = cpool.tile([1, F], F32)
            wrb_t = cpool.tile([1, F], F32)
            ones_row = cpool.tile([1, F], F32)
            nc.sync.dma_start(out=ws_t[:], in_=ws_mat[:])
            nc.sync.dma_start(out=wr_t[:], in_=wr_mat[:])
            nc.sync.dma_start(out=wsb_t[:], in_=wsb[:])
            nc.sync.dma_start(out=wrb_t[:], in_=wrb[:])
            nc.vector.memset(ones_row[:], 1.0)

            def project(src_ap, g, w_t, b_t, dst_ap, dst_row):
                x = spool.tile([128, 4, 128], F32, tag="pp_x")
                nc.sync.dma_start(
                    out=x[:],
                    in_=src_ap[g * 512:(g + 1) * 512, :]
                        .rearrange("(c p) f -> p c f", p=128))
                pT = ppool.tile([128, 4, 128], F32, space="PSUM", tag="pp_t")
                for c in range(4):
                    nc.tensor.transpose(out=pT[:, c, :], in_=x[:, c, :],
                                        identity=ident[:])
                xT = spool.tile([128, 4, 128], F32, tag="pp_xT")
                nc.scalar.copy(xT[:], pT[:])
                pS = ppool.tile([128, 4, 128], F32, space="PSUM", tag="pp_s")
                for c in range(4):
                    nc.tensor.matmul(pS[:, c, :], lhsT=xT[:, c, :], rhs=w_t[:],
                                     start=True, stop=False, skip_group_check=True)
                    nc.tensor.matmul(pS[:, c, :], lhsT=ones_row[:], rhs=b_t[:],
                                     start=False, stop=True, skip_group_check=True)
                y = spool.tile([128, 4, 128], F32, tag="pp_y")
                nc.scalar.copy(y[:], pS[:])
                nc.sync.dma_start(
                    out=dst_ap[dst_row:dst_row + 512, :]
                        .rearrange("(c p) f -> p c f", p=128),
                    in_=y[:])

            for g in range(NP_PAD // 512):
                if g < SPLIT // 512:
                    project(nodes_pad, g, ws_t, wsb_t, tab_lo, g * 512)
                else:
                    project(nodes_pad, g, ws_t, wsb_t, tab_hi, g * 512 - SPLIT)
            for g in range(NL_PAD // 512):
                project(nodes_loc, g, wr_t, wrb_t, tab_r, g * 512)

        tc.strict_bb_all_engine_barrier()

        if debug_phase == "pre":
            # dump first 6272 rows of tab_r into out_d for inspection
            with tc.tile_pool(name="dbg", bufs=2) as dpool:
                for w in range(NWIN):
                    t = dpool.tile([128, 128], F32, tag="dbg_t")
                    nc.sync.dma_start(out=t[:], in_=tab_r[w * 128:(w + 1) * 128, :])
                    nc.sync.dma_start(out=out_d[w * 128:(w + 1) * 128, :], in_=t[:])

        # ---------------- Phase 2: edge stream ----------------
        tabs = {"lo": tab_lo, "hi": tab_hi}
        nblk_run = 0 if debug_phase == "pre" else nblk
        with tc.tile_pool(name="mc", bufs=1) as cpool2, \
             tc.tile_pool(name="stage", bufs=2) as stpool, \
             tc.tile_pool(name="work", bufs=2) as wpool, \
             tc.tile_pool(name="accp", bufs=1) as apool, \
             tc.tile_pool(name="psA", bufs=2, space="PSUM") as psA, \
             tc.tile_pool(name="psD", bufs=2, space="PSUM") as psD:
            iota_t = cpool2.tile([128, 128], F32)
            attn_t = cpool2.tile([128, 128], F32)
            nc.sync.dma_start(out=iota_t[:], in_=iota_in[:])
            nc.sync.dma_start(out=attn_t[:], in_=attn_in[:])
            acc = apool.tile([128, NWIN * 128], F32)
            den_acc = apool.tile([128, NWIN * H], F32)

            agg_ps = None
            den_ps = None
            for b in range(nblk_run):
                sidx_t = stpool.tile([128, BLK * CHUNK // 16], I16, tag="sidx")
                ridx_t = stpool.tile([128, BLK * CHUNK // 16], I16, tag="ridx")
                rloc_t = stpool.tile([128, BLK], F32, tag="rloc")
                nc.sync.dma_start(out=sidx_t[:], in_=sidx_in[b])
                nc.sync.dma_start(out=ridx_t[:], in_=ridx_in[b])
                nc.sync.dma_start(out=rloc_t[:], in_=rloc_in[b])
                s_t = stpool.tile([128, BLK, 128], F32, tag="s_t")
                r_t = stpool.tile([128, BLK, 128], F32, tag="r_t")
                # segment the block's chunks by gather table; cap segments at
                # 8 chunks (1024 indices) -- larger gathers overflow the SWDGE
                # descriptor scratch ring and wedge the device.
                segs = []
                for cc in range(BLK):
                    t = chunk_info(b * BLK + cc)[0]
                    if segs and segs[-1][0] == t and segs[-1][2] < GRP:
                        segs[-1][2] += 1
                    else:
                        segs.append([t, cc, 1])
                for t, cs, nch in segs:
                    nc.gpsimd.dma_gather(
                        out_ap=s_t[:, cs:cs + nch, :], in_ap=tabs[t][:],
                        idxs_ap=sidx_t[:, cs * 8:(cs + nch) * 8],
                        num_idxs=nch * CHUNK, num_idxs_reg=nch * CHUNK,
                        elem_size=F)
                for cs in range(0, BLK, GRP):
                    nc.gpsimd.dma_gather(
                        out_ap=r_t[:, cs:cs + GRP, :], in_ap=tab_r[:],
                        idxs_ap=ridx_t[:, cs * 8:(cs + GRP) * 8],
                        num_idxs=GRP * CHUNK, num_idxs_reg=GRP * CHUNK,
                        elem_size=F)

                if debug_phase == "gather":
                    if b == 0:
                        nc.sync.dma_start(
                            out=out_d[0:BLK * CHUNK, :]
                                .rearrange("(c p) f -> p c f", p=128),
                            in_=s_t[:])
                    continue

                for grp in range(BLK // GRP):
                    c0 = grp * GRP
                    sl = s_t[:, c0:c0 + GRP, :]
                    rl = r_t[:, c0:c0 + GRP, :]
                    x = wpool.tile([128, GRP, 128], F32, tag="x")
                    nc.vector.tensor_add(x[:], sl, rl)
                    u = wpool.tile([128, GRP, 128], F32, tag="u")
                    nc.scalar.activation(u[:], x[:],
                                         mybir.ActivationFunctionType.Exp)
                    w2 = wpool.tile([128, GRP, 128], F32, tag="w2")
                    nc.scalar.activation(w2[:], u[:],
                                         mybir.ActivationFunctionType.Square,
                                         bias=1.0)
                    nc.vector.tensor_scalar_add(w2[:], w2[:], 1.0)
                    rr = wpool.tile([128, GRP, 128], F32, tag="rr")
                    nc.vector.reciprocal_approx_fast(rr[:], w2[:])
                    nc.vector.tensor_scalar(rr[:], rr[:], -2.0, 1.0,
                                            op0=mybir.AluOpType.mult,
                                            op1=mybir.AluOpType.add)
                    h = wpool.tile([128, GRP, 128], F32, tag="h")
                    nc.vector.tensor_tensor(h[:], x[:], rr[:],
                                            op=mybir.AluOpType.mult)
                    nc.vector.tensor_tensor(
                        h[:], h[:],
                        attn_t[:].unsqueeze(1).to_broadcast([128, GRP, 128]),
                        op=mybir.AluOpType.mult)
                    lgt = wpool.tile([128, GRP, H], F32, tag="lgt")
                    nc.vector.tensor_reduce(
                        out=lgt[:].unsqueeze(3),
                        in_=h[:].rearrange("p c (h d) -> p c h d", d=D),
                        op=mybir.AluOpType.add, axis=mybir.AxisListType.X)
                    pT = wpool.tile([128, GRP, H], F32, tag="pT")
                    nc.scalar.activation(pT[:], lgt[:],
                                         mybir.ActivationFunctionType.Exp,
                                         bias=float(attn_bias))
                    msg = wpool.tile([128, GRP, 128], F32, tag="msg")
                    nc.vector.tensor_tensor(
                        msg[:].rearrange("p c (h d) -> p c h d", d=D),
                        sl.rearrange("p c (h d) -> p c h d", d=D),
                        pT[:].unsqueeze(3).to_broadcast([128, GRP, H, D]),
                        op=mybir.AluOpType.mult)
                    oh = wpool.tile([128, GRP, 128], F32, tag="oh")
                    nc.vector.tensor_tensor(
                        oh[:],
                        rloc_t[:, c0:c0 + GRP].unsqueeze(2)
                              .to_broadcast([128, GRP, 128]),
                        iota_t[:].unsqueeze(1).to_broadcast([128, GRP, 128]),
                        op=mybir.AluOpType.is_equal)

                    for cc in range(GRP):
                        g_ch = b * BLK + c0 + cc
                        t, w, pos = chunk_info(g_ch)
                        if w is None:
                            continue
                        if pos == 0:
                            agg_ps = psA.tile([128, 128], F32, space="PSUM",
                                              tag="agg")
                            den_ps = psD.tile([128, H], F32, space="PSUM",
                                              tag="den")
                        first = pos == 0
                        last = pos == cpw - 1
                        nc.tensor.matmul(agg_ps[:], lhsT=oh[:, cc, :],
                                         rhs=msg[:, cc, :], start=first,
                                         stop=last, skip_group_check=True)
                        nc.tensor.matmul(den_ps[:], lhsT=oh[:, cc, :],
                                         rhs=pT[:, cc, :], start=first,
                                         stop=last, skip_group_check=True)
                        if last:
                            nc.scalar.copy(acc[:, w * 128:(w + 1) * 128],
                                           agg_ps[:])
                            nc.scalar.copy(den_acc[:, w * H:(w + 1) * H],
                                           den_ps[:])

            # ---------------- Phase 3: normalize + store ----------------
            if debug_phase not in ("pre", "gather"):
                nc.vector.tensor_scalar_add(den_acc[:], den_acc[:], 1e-30)
                rcp = wpool.tile([128, NWIN * H], F32, tag="rcp")
                scr = wpool.tile([128, NWIN * H], F32, tag="scr")
                nc.vector.reciprocal_approx_accurate(rcp[:], den_acc[:], scr[:])
                outb = wpool.tile([128, NWIN * 128], F32, tag="outb")
                nc.vector.tensor_tensor(
                    outb[:].rearrange("p (w h d) -> p w h d", h=H, d=D),
                    acc[:].rearrange("p (w h d) -> p w h d", h=H, d=D),
                    rcp[:].rearrange("p (w h) -> p w h", h=H).unsqueeze(3)
                          .to_broadcast([128, NWIN, H, D]),
                    op=mybir.AluOpType.mult)
                nc.sync.dma_start(
                    out=out_d[:].rearrange("(w p) f -> p w f", p=128),
                    in_=outb[:].rearrange("p (w f) -> p w f", f=128))

    nc.compile()
    return nc


def _prep_core(senders, receivers, core, lo_ch, hi_ch, nblk):
    """Build sidx/ridx/rloc arrays for one core given the uniform structure."""
    cpw = lo_ch + hi_ch
    e_pad = nblk * BLK * CHUNK
    mask = (receivers // NPC) == core
    s = senders[mask].astype(np.int64)
    r = (receivers[mask] - core * NPC).astype(np.int64)
    win = r // WIN
    half = (s >= SPLIT).astype(np.int64)
    order = np.lexsort((half, win))
    s, r, win, half = s[order], r[order], win[order], half[order]

    sidx_val = np.zeros(e_pad, np.int64)
    ridx_val = np.zeros(e_pad, np.int64)
    rloc_val = np.full(e_pad, 999.0, np.float32)

    # group boundaries for each (win, half)
    key = win * 2 + half
    # destination offset of each group
    for w in range(NWIN):
        for hf in (0, 1):
            gmask = key == (w * 2 + hf)
            n = int(gmask.sum())
            if n == 0:
                continue
            base = (w * cpw + (lo_ch if hf else 0)) * CHUNK
            cap = (hi_ch if hf else lo_ch) * CHUNK
            assert n <= cap, f"window {w} half {hf}: {n} > {cap}"
            sg = s[gmask]
            sidx_val[base:base + n] = sg - (SPLIT if hf else 0)
            ridx_val[base:base + n] = r[gmask]
            rloc_val[base:base + n] = (r[gmask] - w * WIN).astype(np.float32)

    def wrap16(vals):
        # [nblk, 4096] -> idx16[b, 16k+p, s] = vals[b, s*16+p]
        v = vals.reshape(nblk, BLK * CHUNK // 16, 16).astype(np.int16)
        v = np.transpose(v, (0, 2, 1))          # [nblk, 16, 256]
        return np.tile(v, (1, 8, 1)).copy()     # [nblk, 128, 256]

    sidx = wrap16(sidx_val)
    ridx = wrap16(ridx_val)
    rloc = rloc_val.reshape(nblk, BLK, CHUNK).transpose(0, 2, 1).copy()
    return sidx, ridx, rloc


def kernel(nodes, senders, receivers, Ws_k, Ws_b, Wr_k, Wr_b, attn_k, attn_b):
    nodes = np.asarray(nodes, np.float32)
    senders = np.asarray(senders, np.int32)
    receivers = np.asarray(receivers, np.int32)
    assert nodes.shape == (N_NODES, F) and senders.shape == (N_EDGES,)

    # uniform chunk structure across cores
    core_of = receivers // NPC
    r_loc = receivers - core_of * NPC
    win = r_loc // WIN
    half = (senders >= SPLIT).astype(np.int64)
    key = (core_of.astype(np.int64) * NWIN + win) * 2 + half
    counts = np.bincount(key, minlength=NCORE * NWIN * 2).reshape(-1, 2)
    lo_ch = max(1, int(np.ceil(counts[:, 0].max() / CHUNK)))
    hi_ch = max(1, int(np.ceil(counts[:, 1].max() / CHUNK)))
    cpw = lo_ch + hi_ch
    nblk = (NWIN * cpw + BLK - 1) // BLK

    ck = (lo_ch, hi_ch, nblk, float(np.asarray(attn_b).ravel()[0]))
    if ck not in _prog_cache:
        _prog_cache[ck] = _build_program(*ck)
    nc = _prog_cache[ck]

    nodes_pad = np.zeros((NP_PAD, F), np.float32)
    nodes_pad[:N_NODES] = nodes
    ws_mat = np.asarray(Ws_k, np.float32).reshape(F, F)
    wr_mat = np.asarray(Wr_k, np.float32).reshape(F, F)
    wsb = np.asarray(Ws_b, np.float32).reshape(1, F)
    wrb = np.asarray(Wr_b, np.float32).reshape(1, F)
    a_flat = np.tile(np.asarray(attn_k, np.float32).ravel(), H)
    attn_rep = np.broadcast_to(a_flat, (128, 128)).copy()
    iota = np.broadcast_to(np.arange(128, dtype=np.float32), (128, 128)).copy()

    in_maps = []
    for c in range(NCORE):
        sidx, ridx, rloc = _prep_core(senders, receivers, c, lo_ch, hi_ch, nblk)
        nodes_loc = np.zeros((NL_PAD, F), np.float32)
        nodes_loc[:NPC] = nodes[c * NPC:(c + 1) * NPC]
        in_maps.append({
            "nodes_pad": nodes_pad, "nodes_loc": nodes_loc,
            "ws_mat": ws_mat, "wr_mat": wr_mat, "wsb": wsb, "wrb": wrb,
            "iota": iota, "attn_rep": attn_rep,
            "sidx": sidx, "ridx": ridx, "rloc": rloc,
        })

    trace = bool(int(os.environ.get("GAT_TRACE", "0")))
    res = bass_utils.run_bass_kernel_spmd(nc, in_maps,
                                          core_ids=list(range(NCORE)),
                                          trace=trace)
    if trace:
        kernel.last_profile = res
    out = np.empty((N_NODES, F), np.float32)
    for c in range(NCORE):
        out[c * NPC:(c + 1) * NPC] = np.asarray(res.results[c]["out_d"])[:NPC]
    return out



# revision 6
# speedup vs baseline: 1.6838x; 1.6838x over previous
"""GATv2 message passing on 8 Trainium2 NeuronCores (Bass/Tile).

Strategy (edge-parallel by receiver ownership, bf16 compute):
  - Nodes are split into 8 contiguous ranges of 6250; core c owns range c and
    all edges whose receiver falls in it (no cross-core reduction needed).
  - Phase 1: each core projects the full node table with Ws (+Ws_b) and its
    local slice with Wr (+Wr_b) into bf16 DRAM tables.  The node blocks are
    loaded pre-transposed via the HWDGE xbar (dma_start_transpose), so the
    PE only runs the projection matmuls; the bias add rides the PSUM->SBUF
    copy on the vector engine.
  - Phase 2: stream the edge shard sorted by (receiver window, sender>=32768).
    Sender rows come from a bf16 dma_gather (SWDGE, alternating descriptor
    queues so desc-gen overlaps DMA drain).  Receiver rows are NOT gathered:
    a window holds only 128 receiver nodes, so each chunk selects its rows
    from the window-resident r-projection tile with a one-hot matmul
    (ohT = per-partition iota == broadcast rloc).  mish is exp+square on the
    scalar engine plus a reciprocal chain on vector (all bf16); softmax skips
    the max pass (logits are O(1) so exp is safe).  The scatter-add and the
    softmax denominator share one 136-column matmul per chunk into per-window
    PSUM accumulators.
  - out[n] = segsum(exp(logit)*msg) / segsum(exp(logit)), computed on-device;
    host only reassembles the [50000,128] output from the 8 slices.
"""

import os
import sys

for _p in ("/opt/trn_rl_repo", "/root/.axon_site/_ro/trn_rl_repo"):
    if os.path.isdir(_p) and _p not in sys.path:
        sys.path.insert(0, _p)

import numpy as np
import ml_dtypes

import concourse.bass as bass
import concourse.bacc as bacc
import concourse.tile as tile
from concourse import mybir
from concourse import bass_utils

F32 = mybir.dt.float32
BF16 = mybir.dt.bfloat16
I16 = mybir.dt.int16
BF = ml_dtypes.bfloat16

N_NODES = 50000
N_EDGES = 800000
F = 128            # feature dim
H = 8              # heads
D = 16             # head dim
NCORE = 8
NPC = N_NODES // NCORE          # 6250 nodes per core
WIN = 128                       # nodes per scatter window
NWIN = (NPC + WIN - 1) // WIN   # 49 windows per core
SPLIT = 32768                   # int16 gather-index limit -> lo/hi tables
NP_PAD = 50176                  # nodes padded to 98*512 for precompute
NL_PAD = 6656                   # local nodes padded to 13*512
HI_ROWS = NP_PAD - SPLIT        # 17408
CHUNK = 128                     # edges per matmul chunk
UNIT = 4                        # chunks per r-select PSUM tile
BLK = 32                        # chunks per DMA block (4096 edges)
GRP = 8                         # max chunks per dma_gather call
NQ = 4                          # SWDGE descriptor queues (round-robin)

_prog_cache = {}


def _build_program(lo_ch, hi_ch, nblk, attn_bias):
    """Build the SPMD Bass program for chunk structure (lo_ch, hi_ch)."""
    cpw = lo_ch + hi_ch                      # chunks per window
    n_real = NWIN * cpw                      # real chunks in stream

    def chunk_tab(g):
        return "lo" if (g % cpw) < lo_ch else "hi"

    nc = bacc.Bacc("TRN2", target_bir_lowering=False, debug=False,
                   enable_asserts=False, num_devices=NCORE,
                   num_swdge_queues=NQ)

    def dram_in(name, shape, dt=BF16):
        return nc.dram_tensor(name, list(shape), dt, kind="ExternalInput").ap()

    nodes_pad = dram_in("nodes_pad", (NP_PAD, F))
    nodes_loc = dram_in("nodes_loc", (NL_PAD, F))
    ws_mat = dram_in("ws_mat", (F, F))
    wr_mat = dram_in("wr_mat", (F, F))
    wsb_rep = dram_in("wsb_rep", (128, F))
    wrb_rep = dram_in("wrb_rep", (128, F))
    iota_in = dram_in("iota", (128, 128))
    iotac_in = dram_in("iotac", (128, 1), F32)
    attn_in = dram_in("attn_rep", (128, 128))
    sidx_in = dram_in("sidx", (nblk, 128, BLK * CHUNK // 16), I16)
    rloc_in = dram_in("rloc", (nblk, 128, BLK))
    rlocf_in = dram_in("rlocf", (nblk, 1, BLK * CHUNK))
    out_d = nc.dram_tensor("out_d", [NWIN * WIN, F], F32, kind="ExternalOutput").ap()

    tab_lo = nc.dram_tensor("tab_lo", [SPLIT, F], BF16, kind="Internal").ap()
    tab_hi = nc.dram_tensor("tab_hi", [HI_ROWS, F], BF16, kind="Internal").ap()
    tab_r = nc.dram_tensor("tab_r", [NL_PAD, F], BF16, kind="Internal").ap()

    with tile.TileContext(nc) as tc:
        # ---------------- Phase 1: projection precompute ----------------
        with tc.tile_pool(name="pp_const", bufs=1) as cpool, \
             tc.tile_pool(name="pp_sbuf", bufs=3) as spool, \
             tc.tile_pool(name="pp_psum", bufs=2, space="PSUM") as ppool:
            ws_t = cpool.tile([F, F], BF16)
            wr_t = cpool.tile([F, F], BF16)
            wsb_t = cpool.tile([128, F], BF16)
            wrb_t = cpool.tile([128, F], BF16)
            nc.sync.dma_start(out=ws_t[:], in_=ws_mat[:])
            nc.sync.dma_start(out=wr_t[:], in_=wr_mat[:])
            nc.sync.dma_start(out=wsb_t[:], in_=wsb_rep[:])
            nc.sync.dma_start(out=wrb_t[:], in_=wrb_rep[:])

            def project(src_ap, g, w_t, b_t, dst_ap, dst_row):
                xT = spool.tile([128, 4, 128], BF16, tag="pp_x")
                nc.sync.dma_start_transpose(
                    out=xT[:], in_=src_ap[g * 512:(g + 1) * 512, :])
                pS = ppool.tile([128, 4, 128], F32, space="PSUM", tag="pp_s")
                for c in range(4):
                    nc.tensor.matmul(pS[:, c, :], lhsT=xT[:, c, :], rhs=w_t[:],
                                     start=True, stop=True,
                                     skip_group_check=True)
                y = spool.tile([128, 4, 128], BF16, tag="pp_y")
                nc.vector.tensor_tensor(
                    y[:], pS[:],
                    b_t[:].unsqueeze(1).to_broadcast([128, 4, 128]),
                    op=mybir.AluOpType.add)
                nc.sync.dma_start(
                    out=dst_ap[dst_row:dst_row + 512, :]
                        .rearrange("(c p) f -> p c f", p=128),
                    in_=y[:])

            for g in range(NP_PAD // 512):
                if g < SPLIT // 512:
                    project(nodes_pad, g, ws_t, wsb_t, tab_lo, g * 512)
                else:
                    project(nodes_pad, g, ws_t, wsb_t, tab_hi, g * 512 - SPLIT)
            for g in range(NL_PAD // 512):
                project(nodes_loc, g, wr_t, wrb_t, tab_r, g * 512)

        tc.strict_bb_all_engine_barrier()

        # ---------------- Phase 2: edge stream ----------------
        tabs = {"lo": tab_lo, "hi": tab_hi}
        with tc.tile_pool(name="mc", bufs=1) as cpool2, \
             tc.tile_pool(name="stage", bufs=2) as stpool, \
             tc.tile_pool(name="rwinp", bufs=3) as rwpool, \
             tc.tile_pool(name="work", bufs=2) as wpool, \
             tc.tile_pool(name="mid", bufs=1) as mpool, \
             tc.tile_pool(name="accp", bufs=1) as apool, \
             tc.tile_pool(name="psR", bufs=2, space="PSUM") as psR, \
             tc.tile_pool(name="psA", bufs=2, space="PSUM") as psA:
            iota_t = cpool2.tile([128, 128], BF16)
            iotac_t = cpool2.tile([128, 1], F32)
            attn_t = cpool2.tile([128, 128], BF16)
            nc.sync.dma_start(out=iota_t[:], in_=iota_in[:])
            nc.sync.dma_start(out=iotac_t[:], in_=iotac_in[:])
            nc.sync.dma_start(out=attn_t[:], in_=attn_in[:])
            acc = apool.tile([128, NWIN * 128], BF16)
            den_acc = apool.tile([128, NWIN * H], F32)

            acc_ps = None
            rwin_t = None
            qn = [0]

            for b in range(nblk):
                g0 = b * BLK
                rc = min(BLK, n_real - g0)
                if rc <= 0:
                    break
                sidx_t = stpool.tile([128, BLK * CHUNK // 16], I16, tag="sidx")
                rloc_t = stpool.tile([128, BLK], BF16, tag="rloc")
                repf_t = stpool.tile([128, BLK * CHUNK], BF16, tag="repf")
                nc.sync.dma_start(out=sidx_t[:], in_=sidx_in[b])
                nc.sync.dma_start(out=rloc_t[:], in_=rloc_in[b])
                nc.sync.dma_start(
                    out=repf_t[:],
                    in_=rlocf_in[b].to_broadcast([128, BLK * CHUNK]))
                s_t = stpool.tile([128, BLK, 128], BF16, tag="s_t")
                segs = []
                for cc in range(rc):
                    t = chunk_tab(g0 + cc)
                    if segs and segs[-1][0] == t and segs[-1][2] < GRP:
                        segs[-1][2] += 1
                    else:
                        segs.append([t, cc, 1])
                for t, cs, nch in segs:
                    nc.gpsimd.dma_gather(
                        out_ap=s_t[:, cs:cs + nch, :], in_ap=tabs[t][:],
                        idxs_ap=sidx_t[:, cs * 8:(cs + nch) * 8],
                        num_idxs=nch * CHUNK, num_idxs_reg=nch * CHUNK,
                        elem_size=F, queue_num=qn[0])
                    qn[0] = (qn[0] + 1) % NQ

                # one-hot tiles: oh (edges on partitions) for the scatter,
                # ohT (window nodes on partitions) for the r-select matmul.
                ohT_t = wpool.tile([128, BLK * CHUNK], BF16, tag="ohT")
                nc.vector.tensor_scalar(
                    ohT_t[:, :rc * CHUNK], repf_t[:, :rc * CHUNK],
                    iotac_t[:], None, op0=mybir.AluOpType.is_equal)
                oh_t = wpool.tile([128, BLK, 128], BF16, tag="oh")
                nc.vector.tensor_tensor(
                    oh_t[:, :rc, :],
                    rloc_t[:, :rc].unsqueeze(2).to_broadcast([128, rc, 128]),
                    iota_t[:].unsqueeze(1).to_broadcast([128, rc, 128]),
                    op=mybir.AluOpType.is_equal)

                # r-select per unit of 4 chunks; x = s_proj + r_proj
                x_t = wpool.tile([128, BLK, 128], BF16, tag="x")
                nunit = (rc + UNIT - 1) // UNIT
                for u in range(nunit):
                    c0 = u * UNIT
                    cn = min(UNIT, rc - c0)
                    r_ps = psR.tile([128, UNIT, 128], F32, space="PSUM",
                                    tag="r_ps")
                    for j in range(cn):
                        g = g0 + c0 + j
                        if g % cpw == 0:
                            rwin_t = rwpool.tile([128, 128], BF16, tag="rwin")
                            w = g // cpw
                            nc.sync.dma_start(
                                out=rwin_t[:],
                                in_=tab_r[w * 128:(w + 1) * 128, :])
                        nc.tensor.matmul(
                            r_ps[:, j, :],
                            lhsT=ohT_t[:, (c0 + j) * CHUNK:(c0 + j + 1) * CHUNK],
                            rhs=rwin_t[:], start=True, stop=True,
                            skip_group_check=True)
                    nc.vector.tensor_tensor(
                        x_t[:, c0:c0 + cn, :], s_t[:, c0:c0 + cn, :],
                        r_ps[:, :cn, :], op=mybir.AluOpType.add)

                # mish(x) = x * (1 - 2/((1+e^x)^2+1)); then per-head logits
                u_t = mpool.tile([128, BLK, 128], BF16, tag="u")
                nc.scalar.activation(u_t[:, :rc, :], x_t[:, :rc, :],
                                     mybir.ActivationFunctionType.Exp)
                q_t = mpool.tile([128, BLK, 128], F32, tag="q")
                nc.scalar.activation(q_t[:, :rc, :], u_t[:, :rc, :],
                                     mybir.ActivationFunctionType.Square,
                                     bias=1.0)
                nc.vector.tensor_scalar_add(q_t[:, :rc, :], q_t[:, :rc, :], 1.0)
                rr_t = mpool.tile([128, BLK, 128], F32, tag="rr")
                nc.vector.reciprocal_approx_fast(rr_t[:, :rc, :], q_t[:, :rc, :])
                nc.vector.tensor_scalar(rr_t[:, :rc, :], rr_t[:, :rc, :],
                                        -2.0, 1.0,
                                        op0=mybir.AluOpType.mult,
                                        op1=mybir.AluOpType.add)
                h_t = mpool.tile([128, BLK, 128], BF16, tag="h")
                nc.vector.tensor_tensor(h_t[:, :rc, :], x_t[:, :rc, :],
                                        rr_t[:, :rc, :],
                                        op=mybir.AluOpType.mult)
                nc.vector.tensor_tensor(
                    h_t[:, :rc, :], h_t[:, :rc, :],
                    attn_t[:].unsqueeze(1).to_broadcast([128, rc, 128]),
                    op=mybir.AluOpType.mult)
                lgt_t = mpool.tile([128, BLK, H], F32, tag="lgt")
                nc.vector.tensor_reduce(
                    out=lgt_t[:, :rc, :].unsqueeze(3),
                    in_=h_t[:, :rc, :].rearrange("p c (h d) -> p c h d", d=D),
                    op=mybir.AluOpType.add, axis=mybir.AxisListType.X)

                # combined scatter rhs: cols 0..127 = msg, cols 128..135 = pT
                rhs_t = wpool.tile([128, BLK, 136], BF16, tag="rhs")
                nc.scalar.activation(rhs_t[:, :rc, 128:136], lgt_t[:, :rc, :],
                                     mybir.ActivationFunctionType.Exp,
                                     bias=float(attn_bias))
                nc.vector.tensor_tensor(
                    rhs_t[:, :rc, :128].rearrange("p c (h d) -> p c h d", d=D),
                    s_t[:, :rc, :].rearrange("p c (h d) -> p c h d", d=D),
                    rhs_t[:, :rc, 128:136].unsqueeze(3)
                         .to_broadcast([128, rc, H, D]),
                    op=mybir.AluOpType.mult)

                for cc in range(rc):
                    g = g0 + cc
                    w, pos = divmod(g, cpw)
                    if pos == 0:
                        acc_ps = psA.tile([128, 136], F32, space="PSUM",
                                          tag="agg")
                    nc.tensor.matmul(acc_ps[:], lhsT=oh_t[:, cc, :],
                                     rhs=rhs_t[:, cc, :], start=(pos == 0),
                                     stop=(pos == cpw - 1),
                                     skip_group_check=True)
                    if pos == cpw - 1:
                        nc.scalar.activation(
                            acc[:, w * 128:(w + 1) * 128], acc_ps[:, :128],
                            mybir.ActivationFunctionType.Copy)
                        nc.scalar.copy(den_acc[:, w * H:(w + 1) * H],
                                       acc_ps[:, 128:136])

            # ---------------- Phase 3: normalize + store ----------------
            nc.vector.tensor_scalar_add(den_acc[:], den_acc[:], 1e-30)
            rcp = mpool.tile([128, NWIN * H], F32, tag="rcp")
            scr = mpool.tile([128, NWIN * H], F32, tag="scr")
            nc.vector.reciprocal_approx_accurate(rcp[:], den_acc[:], scr[:])
            outb = mpool.tile([128, NWIN * 128], F32, tag="outb")
            nc.vector.tensor_tensor(
                outb[:].rearrange("p (w h d) -> p w h d", h=H, d=D),
                acc[:].rearrange("p (w h d) -> p w h d", h=H, d=D),
                rcp[:].rearrange("p (w h) -> p w h", h=H).unsqueeze(3)
                      .to_broadcast([128, NWIN, H, D]),
                op=mybir.AluOpType.mult)
            nc.sync.dma_start(
                out=out_d[:].rearrange("(w p) f -> p w f", p=128),
                in_=outb[:].rearrange("p (w f) -> p w f", f=128))

    nc.compile()
    return nc


def _prep_core(senders, receivers, core, lo_ch, hi_ch, nblk):
    """Build sidx/rloc/rlocf arrays for one core given the uniform structure."""
    cpw = lo_ch + hi_ch
    e_pad = nblk * BLK * CHUNK
    mask = (receivers // NPC) == core
    s = senders[mask].astype(np.int64)
    r = (receivers[mask] - core * NPC).astype(np.int64)
    win = r // WIN
    half = (s >= SPLIT).astype(np.int64)
    order = np.lexsort((half, win))
    s, r, win, half = s[order], r[order], win[order], half[order]

    sidx_val = np.zeros(e_pad, np.int64)
    rloc_val = np.full(e_pad, 999.0, np.float32)

    key = win * 2 + half
    for w in range(NWIN):
        for hf in (0, 1):
            gmask = key == (w * 2 + hf)
            n = int(gmask.sum())
            if n == 0:
                continue
            base = (w * cpw + (lo_ch if hf else 0)) * CHUNK
            cap = (hi_ch if hf else lo_ch) * CHUNK
            assert n <= cap, f"window {w} half {hf}: {n} > {cap}"
            sg = s[gmask]
            sidx_val[base:base + n] = sg - (SPLIT if hf else 0)
            rloc_val[base:base + n] = (r[gmask] - w * WIN).astype(np.float32)

    def wrap16(vals):
        v = vals.reshape(nblk, BLK * CHUNK // 16, 16).astype(np.int16)
        v = np.transpose(v, (0, 2, 1))          # [nblk, 16, 256]
        return np.tile(v, (1, 8, 1)).copy()     # [nblk, 128, 256]

    sidx = wrap16(sidx_val)
    rloc = rloc_val.reshape(nblk, BLK, CHUNK).transpose(0, 2, 1).astype(BF).copy()
    rlocf = rloc_val.reshape(nblk, 1, BLK * CHUNK).astype(BF).copy()
    return sidx, rloc, rlocf


def kernel(nodes, senders, receivers, Ws_k, Ws_b, Wr_k, Wr_b, attn_k, attn_b):
    nodes = np.asarray(nodes, np.float32)
    senders = np.asarray(senders, np.int32)
    receivers = np.asarray(receivers, np.int32)
    assert nodes.shape == (N_NODES, F) and senders.shape == (N_EDGES,)

    # uniform chunk structure across cores
    core_of = receivers // NPC
    r_loc = receivers - core_of * NPC
    win = r_loc // WIN
    half = (senders >= SPLIT).astype(np.int64)
    key = (core_of.astype(np.int64) * NWIN + win) * 2 + half
    counts = np.bincount(key, minlength=NCORE * NWIN * 2).reshape(-1, 2)
    lo_ch = max(1, int(np.ceil(counts[:, 0].max() / CHUNK)))
    hi_ch = max(1, int(np.ceil(counts[:, 1].max() / CHUNK)))
    cpw = lo_ch + hi_ch
    nblk = (NWIN * cpw + BLK - 1) // BLK

    ck = (lo_ch, hi_ch, nblk, float(np.asarray(attn_b).ravel()[0]))
    if ck not in _prog_cache:
        _prog_cache[ck] = _build_program(*ck)
    nc = _prog_cache[ck]

    nodes_bf = nodes.astype(BF)
    nodes_pad = np.zeros((NP_PAD, F), BF)
    nodes_pad[:N_NODES] = nodes_bf
    ws_mat = np.asarray(Ws_k, np.float32).reshape(F, F).astype(BF)
    wr_mat = np.asarray(Wr_k, np.float32).reshape(F, F).astype(BF)
    wsb_rep = np.broadcast_to(
        np.asarray(Ws_b, np.float32).reshape(1, F), (128, F)).astype(BF).copy()
    wrb_rep = np.broadcast_to(
        np.asarray(Wr_b, np.float32).reshape(1, F), (128, F)).astype(BF).copy()
    a_flat = np.tile(np.asarray(attn_k, np.float32).ravel(), H)
    attn_rep = np.broadcast_to(a_flat, (128, 128)).astype(BF).copy()
    iota = np.broadcast_to(np.arange(128, dtype=np.float32),
                           (128, 128)).astype(BF).copy()
    iotac = np.arange(128, dtype=np.float32)[:, None].copy()

    in_maps = []
    for c in range(NCORE):
        sidx, rloc, rlocf = _prep_core(senders, receivers, c, lo_ch, hi_ch,
                                       nblk)
        nodes_loc = np.zeros((NL_PAD, F), BF)
        nodes_loc[:NPC] = nodes_bf[c * NPC:(c + 1) * NPC]
        in_maps.append({
            "nodes_pad": nodes_pad, "nodes_loc": nodes_loc,
            "ws_mat": ws_mat, "wr_mat": wr_mat,
            "wsb_rep": wsb_rep, "wrb_rep": wrb_rep,
            "iota": iota, "iotac": iotac, "attn_rep": attn_rep,
            "sidx": sidx, "rloc": rloc, "rlocf": rlocf,
        })

    trace = bool(int(os.environ.get("GAT_TRACE", "0")))
    res = bass_utils.run_bass_kernel_spmd(nc, in_maps,
                                          core_ids=list(range(NCORE)),
                                          trace=trace)
    if trace:
        kernel.last_profile = res
    out = np.empty((N_NODES, F), np.float32)
    for c in range(NCORE):
        out[c * NPC:(c + 1) * NPC] = np.asarray(res.results[c]["out_d"])[:NPC]
    return out


# revision 12
# speedup vs baseline: 2.3397x; 1.3895x over previous
"""GATv2 message passing on 8 Trainium2 NeuronCores (Bass/Tile).

Strategy (edge-parallel by receiver ownership, bf16 compute):
  - Nodes are split into 8 contiguous ranges of 6250; core c owns range c and
    all edges whose receiver falls in it (no cross-core reduction needed).
  - Phase 1: each core projects the full node table with Ws (+Ws_b) and its
    local slice with Wr (+Wr_b) into bf16 DRAM tables.  2048-row groups are
    loaded pre-transposed through the HWDGE xbar (dma_start_transpose), the
    PE runs the projection matmuls, the bias add rides the PSUM->SBUF copy
    on the vector engine, and the table write-back goes out on the scalar
    engine's HWDGE queue so loads and stores issue in parallel.
  - Phase 2: stream the edge shard sorted by (receiver window, sender>=32768).
    Sender rows come from a bf16 dma_gather (SWDGE, 4 descriptor queues
    round-robin so desc-gen overlaps DMA drain).  Receiver rows are NOT
    gathered: a window holds only 128 receiver nodes, so each chunk selects
    its rows from the window-resident r-projection tile with a one-hot
    matmul; the sender rows are accumulated into the same PSUM via an
    identity matmul, so mish reads x = s_proj + r_proj straight from PSUM.
    mish itself is a single scalar-engine activation (the Mish table);
    logits finish with a pairwise-add reduce tree (2x DVE mode) and the
    softmax weight is expanded head->features on the scalar engine so the
    msg multiply also runs in 2x mode.  Softmax skips the max pass (logits
    are O(1) so exp is safe in f32/bf16 range).  The scatter-add and the
    softmax denominator share one 136-column matmul per chunk into
    per-window PSUM accumulators.
  - out[n] = segsum(exp(logit)*msg) / segsum(exp(logit)), computed on-device;
    host only reassembles the [50000,128] output from the 8 slices.
"""

import os
import sys

for _p in ("/opt/trn_rl_repo", "/root/.axon_site/_ro/trn_rl_repo"):
    if os.path.isdir(_p) and _p not in sys.path:
        sys.path.insert(0, _p)

import numpy as np
import ml_dtypes

import concourse.bass as bass
import concourse.bacc as bacc
import concourse.tile as tile
from concourse import mybir
from concourse import bass_utils
from concourse.masks import make_identity

F32 = mybir.dt.float32
BF16 = mybir.dt.bfloat16
I16 = mybir.dt.int16
BF = ml_dtypes.bfloat16

N_NODES = 50000
N_EDGES = 800000
F = 128            # feature dim
H = 8              # heads
D = 16             # head dim
NCORE = 8
NPC = N_NODES // NCORE          # 6250 nodes per core
WIN = 128                       # nodes per scatter window
NWIN = (NPC + WIN - 1) // WIN   # 49 windows per core
SPLIT = 32768                   # int16 gather-index limit -> lo/hi tables
NP_PAD = 50176                  # nodes padded to 98*512 for precompute
NL_PAD = 6656                   # local nodes padded to 13*512
HI_ROWS = NP_PAD - SPLIT        # 17408
CHUNK = 128                     # edges per matmul chunk
UNIT = 4                        # chunks per r-select PSUM tile
BLK = 32                        # chunks per DMA block (4096 edges)
GRP = 8                         # max chunks per dma_gather call
NQ = 4                          # SWDGE descriptor queues (round-robin)

_prog_cache = {}


def _build_program(lo_ch, hi_ch, nblk, attn_bias):
    """Build the SPMD Bass program for chunk structure (lo_ch, hi_ch)."""
    cpw = lo_ch + hi_ch                      # chunks per window
    n_real = NWIN * cpw                      # real chunks in stream

    def chunk_tab(g):
        return "lo" if (g % cpw) < lo_ch else "hi"

    nc = bacc.Bacc("TRN2", target_bir_lowering=False, debug=False,
                   enable_asserts=False, num_devices=NCORE,
                   num_swdge_queues=NQ)

    def dram_in(name, shape, dt=BF16):
        return nc.dram_tensor(name, list(shape), dt, kind="ExternalInput").ap()

    nodes_pad = dram_in("nodes_pad", (NP_PAD, F))
    nodes_loc = dram_in("nodes_loc", (NL_PAD, F))
    ws_mat = dram_in("ws_mat", (F, F))
    wr_mat = dram_in("wr_mat", (F, F))
    wsb_rep = dram_in("wsb_rep", (128, F))
    wrb_rep = dram_in("wrb_rep", (128, F))
    iotac_in = dram_in("iotac", (128, 1), F32)
    attn_in = dram_in("attn_rep", (128, 128))
    sidx_in = dram_in("sidx", (nblk, 128, BLK * CHUNK // 16), I16)
    ohh_in = dram_in("ohh", (nblk, 128, BLK * CHUNK))
    rlocf_in = dram_in("rlocf", (nblk, 1, BLK * CHUNK))
    out_d = nc.dram_tensor("out_d", [NWIN * WIN, F], F32, kind="ExternalOutput").ap()

    tab_lo = nc.dram_tensor("tab_lo", [SPLIT, F], BF16, kind="Internal").ap()
    tab_hi = nc.dram_tensor("tab_hi", [HI_ROWS, F], BF16, kind="Internal").ap()
    tab_r = nc.dram_tensor("tab_r", [NL_PAD, F], BF16, kind="Internal").ap()

    with tile.TileContext(nc) as tc:
        # ---------------- Phase 1: projection precompute ----------------
        with tc.tile_pool(name="pp_const", bufs=1) as cpool, \
             tc.tile_pool(name="pp_sbuf", bufs=3) as spool, \
             tc.tile_pool(name="pp_psum", bufs=2, space="PSUM") as ppool:
            ws_t = cpool.tile([F, F], BF16)
            wr_t = cpool.tile([F, F], BF16)
            wsb_t = cpool.tile([128, F], BF16)
            wrb_t = cpool.tile([128, F], BF16)
            nc.sync.dma_start(out=ws_t[:], in_=ws_mat[:])
            nc.sync.dma_start(out=wr_t[:], in_=wr_mat[:])
            nc.sync.dma_start(out=wsb_t[:], in_=wsb_rep[:])
            nc.sync.dma_start(out=wrb_t[:], in_=wrb_rep[:])

            def project(src_ap, src_row, nrows, w_t, b_t, dst_ap, dst_row):
                nch = nrows // 128
                xT = spool.tile([128, nch, 128], BF16, tag=f"pp_x{nrows}")
                nc.sync.dma_start_transpose(
                    out=xT[:], in_=src_ap[src_row:src_row + nrows, :])
                y = spool.tile([128, nch, 128], BF16, tag=f"pp_y{nrows}")
                for sub in range(nch // 4):
                    pS = ppool.tile([128, 4, 128], F32, space="PSUM",
                                    tag="pp_s")
                    for c in range(4):
                        nc.tensor.matmul(pS[:, c, :],
                                         lhsT=xT[:, sub * 4 + c, :],
                                         rhs=w_t[:], start=True, stop=True,
                                         skip_group_check=True)
                    nc.vector.tensor_tensor(
                        y[:, sub * 4:(sub + 1) * 4, :], pS[:],
                        b_t[:].unsqueeze(1).to_broadcast([128, 4, 128]),
                        op=mybir.AluOpType.add)
                nc.scalar.dma_start(
                    out=dst_ap[dst_row:dst_row + nrows, :]
                        .rearrange("(c p) f -> p c f", p=128),
                    in_=y[:])

            # tab_lo: rows 0..32767 of nodes_pad
            for row in range(0, SPLIT, 2048):
                project(nodes_pad, row, 2048, ws_t, wsb_t, tab_lo, row)
            # tab_hi: rows 32768..50175
            row = SPLIT
            while row < NP_PAD:
                nrows = min(2048, NP_PAD - row)
                project(nodes_pad, row, nrows, ws_t, wsb_t, tab_hi,
                        row - SPLIT)
                row += nrows
            # tab_r: local nodes
            row = 0
            while row < NL_PAD:
                nrows = min(2048, NL_PAD - row)
                project(nodes_loc, row, nrows, wr_t, wrb_t, tab_r, row)
                row += nrows

        tc.strict_bb_all_engine_barrier()

        # ---------------- Phase 2: edge stream ----------------
        tabs = {"lo": tab_lo, "hi": tab_hi}
        with tc.tile_pool(name="mc", bufs=1) as cpool2, \
             tc.tile_pool(name="stage", bufs=2) as stpool, \
             tc.tile_pool(name="rwinp", bufs=3) as rwpool, \
             tc.tile_pool(name="work", bufs=2) as wpool, \
             tc.tile_pool(name="mid", bufs=1) as mpool, \
             tc.tile_pool(name="psR", bufs=3, space="PSUM") as psR, \
             tc.tile_pool(name="psA", bufs=2, space="PSUM") as psA:
            iotac_t = cpool2.tile([128, 1], F32)
            attn_t = cpool2.tile([128, 128], BF16)
            ident_t = cpool2.tile([128, 128], BF16)
            nc.sync.dma_start(out=iotac_t[:], in_=iotac_in[:])
            nc.sync.dma_start(out=attn_t[:], in_=attn_in[:])
            make_identity(nc, ident_t[:])

            acc_ps = None
            rwin_t = None
            qn = [0]

            for b in range(nblk):
                g0 = b * BLK
                rc = min(BLK, n_real - g0)
                if rc <= 0:
                    break
                sidx_t = stpool.tile([128, BLK * CHUNK // 16], I16, tag="sidx")
                repf_t = stpool.tile([128, BLK * CHUNK], BF16, tag="repf")
                oh_t = stpool.tile([128, BLK * CHUNK], BF16, tag="oh")
                nc.sync.dma_start(out=sidx_t[:], in_=sidx_in[b])
                nc.sync.dma_start(
                    out=repf_t[:],
                    in_=rlocf_in[b].to_broadcast([128, BLK * CHUNK]))
                nc.sync.dma_start(out=oh_t[:], in_=ohh_in[b])
                s_t = stpool.tile([128, BLK, 128], BF16, tag="s_t")
                segs = []
                for cc in range(rc):
                    t = chunk_tab(g0 + cc)
                    if segs and segs[-1][0] == t and segs[-1][2] < GRP:
                        segs[-1][2] += 1
                    else:
                        segs.append([t, cc, 1])
                for t, cs, nchk in segs:
                    nc.gpsimd.dma_gather(
                        out_ap=s_t[:, cs:cs + nchk, :], in_ap=tabs[t][:],
                        idxs_ap=sidx_t[:, cs * 8:(cs + nchk) * 8],
                        num_idxs=nchk * CHUNK, num_idxs_reg=nchk * CHUNK,
                        elem_size=F, queue_num=qn[0])
                    qn[0] = (qn[0] + 1) % NQ

                # ohT: window nodes on partitions (for the r-select matmul)
                ohT_t = wpool.tile([128, BLK * CHUNK], BF16, tag="ohT")
                nc.vector.tensor_scalar(
                    ohT_t[:, :rc * CHUNK], repf_t[:, :rc * CHUNK],
                    iotac_t[:], None, op0=mybir.AluOpType.is_equal)

                # per-unit: PSUM x = s_proj (identity mm) + r_proj (one-hot
                # mm against the window tile); x copied out on the scalar
                # engine so DVE ops downstream all run on bf16 SBUF tiles
                x_t = wpool.tile([128, BLK, 128], BF16, tag="x")
                nunit = (rc + UNIT - 1) // UNIT
                for u in range(nunit):
                    c0 = u * UNIT
                    cn = min(UNIT, rc - c0)
                    r_ps = psR.tile([128, UNIT, 128], F32, space="PSUM",
                                    tag="r_ps")
                    nc.tensor.matmul(r_ps[:, :cn, :], lhsT=ident_t[:],
                                     rhs=s_t[:, c0:c0 + cn, :], start=True,
                                     stop=False, skip_group_check=True)
                    for j in range(cn):
                        g = g0 + c0 + j
                        if g % cpw == 0:
                            rwin_t = rwpool.tile([128, 128], BF16, tag="rwin")
                            w = g // cpw
                            nc.sync.dma_start(
                                out=rwin_t[:],
                                in_=tab_r[w * 128:(w + 1) * 128, :])
                        nc.tensor.matmul(
                            r_ps[:, j, :],
                            lhsT=ohT_t[:, (c0 + j) * CHUNK:(c0 + j + 1) * CHUNK],
                            rhs=rwin_t[:], start=False, stop=(j == cn - 1),
                            skip_group_check=True)
                    nc.scalar.activation(x_t[:, c0:c0 + cn, :],
                                         r_ps[:, :cn, :],
                                         mybir.ActivationFunctionType.Copy)

                # mish(x) = x * (1 - 2/((1+e^x)^2+1)): exp+square on the
                # scalar engine (both in the exp_and_others table set), the
                # reciprocal chain on DVE in f32, everything else bf16 2x
                u_t = mpool.tile([128, BLK, 128], BF16, tag="u")
                nc.scalar.activation(u_t[:, :rc, :], x_t[:, :rc, :],
                                     mybir.ActivationFunctionType.Exp)
                q_t = mpool.tile([128, BLK, 128], F32, tag="q")
                nc.scalar.activation(q_t[:, :rc, :], u_t[:, :rc, :],
                                     mybir.ActivationFunctionType.Square,
                                     bias=1.0)
                nc.vector.tensor_scalar_add(q_t[:, :rc, :], q_t[:, :rc, :],
                                            1.0)
                rcp_t = mpool.tile([128, BLK, 128], F32, tag="rcp")
                nc.vector.reciprocal_approx_fast(rcp_t[:, :rc, :],
                                                 q_t[:, :rc, :])
                rr_t = mpool.tile([128, BLK, 128], BF16, tag="rr")
                nc.vector.tensor_scalar(rr_t[:, :rc, :], rcp_t[:, :rc, :],
                                        -2.0, 1.0,
                                        op0=mybir.AluOpType.mult,
                                        op1=mybir.AluOpType.add)
                h_t = mpool.tile([128, BLK, 128], BF16, tag="h")
                nc.vector.tensor_tensor(h_t[:, :rc, :], x_t[:, :rc, :],
                                        rr_t[:, :rc, :],
                                        op=mybir.AluOpType.mult)

                # per-head logits: ha = mish * attn, then pairwise-add tree
                nc.vector.tensor_tensor(
                    h_t[:, :rc, :], h_t[:, :rc, :],
                    attn_t[:].unsqueeze(1).to_broadcast([128, rc, 128]),
                    op=mybir.AluOpType.mult)
                hv = h_t[:, :rc, :].rearrange("p c (h d) -> p c h d", d=D)
                t8 = mpool.tile([128, BLK, H, 8], BF16, tag="t8")
                nc.vector.tensor_tensor(t8[:, :rc], hv[:, :, :, 0:8],
                                        hv[:, :, :, 8:16],
                                        op=mybir.AluOpType.add)
                t4 = mpool.tile([128, BLK, H, 4], BF16, tag="t4")
                nc.vector.tensor_tensor(t4[:, :rc], t8[:, :rc, :, 0:4],
                                        t8[:, :rc, :, 4:8],
                                        op=mybir.AluOpType.add)
                t2 = mpool.tile([128, BLK, H, 2], BF16, tag="t2")
                nc.vector.tensor_tensor(t2[:, :rc], t4[:, :rc, :, 0:2],
                                        t4[:, :rc, :, 2:4],
                                        op=mybir.AluOpType.add)
                lgt_t = mpool.tile([128, BLK, H], BF16, tag="lgt")
                nc.vector.tensor_tensor(lgt_t[:, :rc, :].unsqueeze(3),
                                        t2[:, :rc, :, 0:1], t2[:, :rc, :, 1:2],
                                        op=mybir.AluOpType.add)

                # softmax weight, expanded head->features on the scalar
                # engine so msg runs in 2x DVE mode
                pT128_t = wpool.tile([128, BLK, 128], BF16, tag="pT128")
                nc.scalar.activation(
                    pT128_t[:, :rc, :].rearrange("p c (h d) -> p c h d", d=D),
                    lgt_t[:, :rc, :].unsqueeze(3).to_broadcast([128, rc, H, D]),
                    mybir.ActivationFunctionType.Exp, bias=float(attn_bias))
                rhs_t = wpool.tile([128, BLK, 136], BF16, tag="rhs")
                nc.scalar.activation(rhs_t[:, :rc, 128:136], lgt_t[:, :rc, :],
                                     mybir.ActivationFunctionType.Exp,
                                     bias=float(attn_bias))
                nc.vector.tensor_tensor(rhs_t[:, :rc, :128], s_t[:, :rc, :],
                                        pT128_t[:, :rc, :],
                                        op=mybir.AluOpType.mult)

                for cc in range(rc):
                    g = g0 + cc
                    w, pos = divmod(g, cpw)
                    if pos == 0:
                        acc_ps = psA.tile([128, 136], F32, space="PSUM",
                                          tag="agg")
                    nc.tensor.matmul(acc_ps[:],
                                     lhsT=oh_t[:, cc * CHUNK:(cc + 1) * CHUNK],
                                     rhs=rhs_t[:, cc, :], start=(pos == 0),
                                     stop=(pos == cpw - 1),
                                     skip_group_check=True)
                    if pos == cpw - 1:
                        # normalize straight out of PSUM and store the window
                        denw = mpool.tile([128, H], F32, tag="denw")
                        nc.vector.tensor_scalar_add(denw[:],
                                                    acc_ps[:, 128:136], 1e-30)
                        rcpw = mpool.tile([128, H], F32, tag="rcpw")
                        nc.vector.reciprocal_approx_fast(rcpw[:], denw[:])
                        outw = mpool.tile([128, 128], F32, tag="outw")
                        nc.vector.tensor_tensor(
                            outw[:].rearrange("p (h d) -> p h d", d=D),
                            acc_ps[:, :128].rearrange("p (h d) -> p h d", d=D),
                            rcpw[:].unsqueeze(2).to_broadcast([128, H, D]),
                            op=mybir.AluOpType.mult)
                        nc.sync.dma_start(
                            out=out_d[w * 128:(w + 1) * 128, :], in_=outw[:])

    nc.compile()
    return nc


def _prep_core(senders, receivers, core, lo_ch, hi_ch, nblk):
    """Build sidx/ohh/rlocf arrays for one core given the uniform structure."""
    cpw = lo_ch + hi_ch
    e_pad = nblk * BLK * CHUNK
    mask = (receivers // NPC) == core
    s = senders[mask].astype(np.int64)
    r = (receivers[mask] - core * NPC).astype(np.int64)
    win = r // WIN
    half = (s >= SPLIT).astype(np.int64)
    order = np.lexsort((half, win))
    s, r, win, half = s[order], r[order], win[order], half[order]

    sidx_val = np.zeros(e_pad, np.int64)
    rloc_val = np.full(e_pad, 999.0, np.float32)

    key = win * 2 + half
    for w in range(NWIN):
        for hf in (0, 1):
            gmask = key == (w * 2 + hf)
            n = int(gmask.sum())
            if n == 0:
                continue
            base = (w * cpw + (lo_ch if hf else 0)) * CHUNK
            cap = (hi_ch if hf else lo_ch) * CHUNK
            assert n <= cap, f"window {w} half {hf}: {n} > {cap}"
            sg = s[gmask]
            sidx_val[base:base + n] = sg - (SPLIT if hf else 0)
            rloc_val[base:base + n] = (r[gmask] - w * WIN).astype(np.float32)

    def wrap16(vals):
        v = vals.reshape(nblk, BLK * CHUNK // 16, 16).astype(np.int16)
        v = np.transpose(v, (0, 2, 1))          # [nblk, 16, 256]
        return np.tile(v, (1, 8, 1)).copy()     # [nblk, 128, 256]

    sidx = wrap16(sidx_val)
    rv = rloc_val.reshape(nblk, BLK, CHUNK)     # [b, c, p]
    oh = (rv[:, :, :, None] == np.arange(128, dtype=np.float32))
    ohh = np.ascontiguousarray(
        oh.transpose(0, 2, 1, 3)).reshape(nblk, CHUNK, BLK * 128)
    ohh = ohh.astype(np.float32).astype(BF)
    rlocf = rloc_val.reshape(nblk, 1, BLK * CHUNK).astype(BF).copy()
    return sidx, ohh, rlocf


def kernel(nodes, senders, receivers, Ws_k, Ws_b, Wr_k, Wr_b, attn_k, attn_b):
    nodes = np.asarray(nodes, np.float32)
    senders = np.asarray(senders, np.int32)
    receivers = np.asarray(receivers, np.int32)
    assert nodes.shape == (N_NODES, F) and senders.shape == (N_EDGES,)

    # uniform chunk structure across cores
    core_of = receivers // NPC
    r_loc = receivers - core_of * NPC
    win = r_loc // WIN
    half = (senders >= SPLIT).astype(np.int64)
    key = (core_of.astype(np.int64) * NWIN + win) * 2 + half
    counts = np.bincount(key, minlength=NCORE * NWIN * 2).reshape(-1, 2)
    lo_ch = max(1, int(np.ceil(counts[:, 0].max() / CHUNK)))
    hi_ch = max(1, int(np.ceil(counts[:, 1].max() / CHUNK)))
    cpw = lo_ch + hi_ch
    nblk = (NWIN * cpw + BLK - 1) // BLK

    ck = (lo_ch, hi_ch, nblk, float(np.asarray(attn_b).ravel()[0]))
    if ck not in _prog_cache:
        _prog_cache[ck] = _build_program(*ck)
    nc = _prog_cache[ck]

    nodes_bf = nodes.astype(BF)
    nodes_pad = np.zeros((NP_PAD, F), BF)
    nodes_pad[:N_NODES] = nodes_bf
    ws_mat = np.asarray(Ws_k, np.float32).reshape(F, F).astype(BF)
    wr_mat = np.asarray(Wr_k, np.float32).reshape(F, F).astype(BF)
    wsb_rep = np.broadcast_to(
        np.asarray(Ws_b, np.float32).reshape(1, F), (128, F)).astype(BF).copy()
    wrb_rep = np.broadcast_to(
        np.asarray(Wr_b, np.float32).reshape(1, F), (128, F)).astype(BF).copy()
    a_flat = np.tile(np.asarray(attn_k, np.float32).ravel(), H)
    attn_rep = np.broadcast_to(a_flat, (128, 128)).astype(BF).copy()
    iotac = np.arange(128, dtype=np.float32)[:, None].copy()

    in_maps = []
    for c in range(NCORE):
        sidx, ohh, rlocf = _prep_core(senders, receivers, c, lo_ch, hi_ch,
                                      nblk)
        nodes_loc = np.zeros((NL_PAD, F), BF)
        nodes_loc[:NPC] = nodes_bf[c * NPC:(c + 1) * NPC]
        in_maps.append({
            "nodes_pad": nodes_pad, "nodes_loc": nodes_loc,
            "ws_mat": ws_mat, "wr_mat": wr_mat,
            "wsb_rep": wsb_rep, "wrb_rep": wrb_rep,
            "iotac": iotac, "attn_rep": attn_rep,
            "sidx": sidx, "ohh": ohh, "rlocf": rlocf,
        })

    trace = bool(int(os.environ.get("GAT_TRACE", "0")))
    res = bass_utils.run_bass_kernel_spmd(nc, in_maps,
                                          core_ids=list(range(NCORE)),
                                          trace=trace)
    if trace:
        kernel.last_profile = res
    out = np.empty((N_NODES, F), np.float32)
    for c in range(NCORE):
        out[c * NPC:(c + 1) * NPC] = np.asarray(res.results[c]["out_d"])[:NPC]
    return out


# revision 13
# speedup vs baseline: 2.3908x; 1.0219x over previous
"""GATv2 message passing on 8 Trainium2 NeuronCores (Bass/Tile).

Strategy (edge-parallel by receiver ownership, bf16 compute):
  - Nodes are split into 8 contiguous ranges of 6250; core c owns range c and
    all edges whose receiver falls in it (no cross-core reduction needed).
  - Phase 1: each core projects the full node table with Ws (+Ws_b) and its
    local slice with Wr (+Wr_b) into bf16 DRAM tables.  2048-row groups are
    loaded pre-transposed through the HWDGE xbar (dma_start_transpose), the
    PE runs the projection matmuls, the bias add rides the PSUM->SBUF copy
    on the vector engine, and the table write-back goes out on the scalar
    engine's HWDGE queue so loads and stores issue in parallel.
  - Phase 2: stream the edge shard sorted by (receiver window, sender>=32768),
    with a per-window chunk budget set by the max count across the 8 cores
    (the SPMD program is data-dependent; the compile is cached per graph
    structure).  Sender rows come from a bf16 dma_gather (SWDGE, 4 descriptor
    queues round-robin so desc-gen overlaps DMA drain).  Receiver rows are
    NOT gathered: a window holds only 128 receiver nodes, so each chunk
    selects its rows from the window-resident r-projection tile with a
    one-hot matmul; the sender rows are accumulated into the same PSUM via
    an identity matmul, so x = s_proj + r_proj comes out of PSUM with one
    scalar-engine copy.  mish is exp+square on the scalar engine (both live
    in the exp_and_others activation table, so no table reloads) plus an
    approximate-reciprocal chain on DVE; logits finish with a pairwise-add
    reduce tree (2x DVE mode) and the softmax weight is expanded
    head->features on the scalar engine so the msg multiply also runs in 2x
    mode.  Softmax skips the max pass (logits are O(1) so exp is safe).
    The scatter-add and the softmax denominator share one 136-column matmul
    per chunk into per-window PSUM accumulators, and each window is
    normalized straight out of PSUM and stored.
  - out[n] = segsum(exp(logit)*msg) / segsum(exp(logit)), computed on-device;
    host only reassembles the [50000,128] output from the 8 slices.
"""

import os
import sys

for _p in ("/opt/trn_rl_repo", "/root/.axon_site/_ro/trn_rl_repo"):
    if os.path.isdir(_p) and _p not in sys.path:
        sys.path.insert(0, _p)

import numpy as np
import ml_dtypes

import concourse.bass as bass
import concourse.bacc as bacc
import concourse.tile as tile
from concourse import mybir
from concourse import bass_utils
from concourse.masks import make_identity

F32 = mybir.dt.float32
BF16 = mybir.dt.bfloat16
I16 = mybir.dt.int16
BF = ml_dtypes.bfloat16

N_NODES = 50000
N_EDGES = 800000
F = 128            # feature dim
H = 8              # heads
D = 16             # head dim
NCORE = 8
NPC = N_NODES // NCORE          # 6250 nodes per core
WIN = 128                       # nodes per scatter window
NWIN = (NPC + WIN - 1) // WIN   # 49 windows per core
SPLIT = 32768                   # int16 gather-index limit -> lo/hi tables
NP_PAD = 50176                  # nodes padded to 98*512 for precompute
NL_PAD = 6656                   # local nodes padded to 13*512
HI_ROWS = NP_PAD - SPLIT        # 17408
CHUNK = 128                     # edges per matmul chunk
UNIT = 4                        # chunks per r-select PSUM tile
BLK = 32                        # chunks per DMA block (4096 edges)
GRP = 8                         # max chunks per dma_gather call
NQ = 4                          # SWDGE descriptor queues (round-robin)

_prog_cache = {}


def _chunk_meta(wstruct):
    """Per-chunk (window, table, first, last) from per-window chunk counts."""
    meta = []
    for w, (lw, hw) in enumerate(wstruct):
        cw = lw + hw
        for pos in range(cw):
            meta.append((w, "lo" if pos < lw else "hi",
                         pos == 0, pos == cw - 1))
    return meta


def _build_program(wstruct, nblk, attn_bias):
    """Build the SPMD Bass program for the per-window chunk structure."""
    meta = _chunk_meta(wstruct)
    n_real = len(meta)
    assert nblk * BLK >= n_real

    nc = bacc.Bacc("TRN2", target_bir_lowering=False, debug=False,
                   enable_asserts=False, num_devices=NCORE,
                   num_swdge_queues=NQ)

    def dram_in(name, shape, dt=BF16):
        return nc.dram_tensor(name, list(shape), dt, kind="ExternalInput").ap()

    nodes_pad = dram_in("nodes_pad", (NP_PAD, F))
    nodes_loc = dram_in("nodes_loc", (NL_PAD, F))
    ws_mat = dram_in("ws_mat", (F, F))
    wr_mat = dram_in("wr_mat", (F, F))
    wsb_rep = dram_in("wsb_rep", (128, F))
    wrb_rep = dram_in("wrb_rep", (128, F))
    iota_in = dram_in("iota", (128, 128))
    iotac_in = dram_in("iotac", (128, 1), F32)
    attn_in = dram_in("attn_rep", (128, 128))
    sidx_in = dram_in("sidx", (nblk, 128, BLK * CHUNK // 16), I16)
    rloc_in = dram_in("rloc", (nblk, 128, BLK), F32)
    rlocf_in = dram_in("rlocf", (nblk, 1, BLK * CHUNK))
    out_d = nc.dram_tensor("out_d", [NWIN * WIN, F], F32, kind="ExternalOutput").ap()

    tab_lo = nc.dram_tensor("tab_lo", [SPLIT, F], BF16, kind="Internal").ap()
    tab_hi = nc.dram_tensor("tab_hi", [HI_ROWS, F], BF16, kind="Internal").ap()
    tab_r = nc.dram_tensor("tab_r", [NL_PAD, F], BF16, kind="Internal").ap()

    with tile.TileContext(nc) as tc:
        # ---------------- Phase 1: projection precompute ----------------
        with tc.tile_pool(name="pp_const", bufs=1) as cpool, \
             tc.tile_pool(name="pp_sbuf", bufs=3) as spool, \
             tc.tile_pool(name="pp_psum", bufs=2, space="PSUM") as ppool:
            ws_t = cpool.tile([F, F], BF16)
            wr_t = cpool.tile([F, F], BF16)
            wsb_t = cpool.tile([128, F], BF16)
            wrb_t = cpool.tile([128, F], BF16)
            nc.sync.dma_start(out=ws_t[:], in_=ws_mat[:])
            nc.sync.dma_start(out=wr_t[:], in_=wr_mat[:])
            nc.sync.dma_start(out=wsb_t[:], in_=wsb_rep[:])
            nc.sync.dma_start(out=wrb_t[:], in_=wrb_rep[:])

            def project(src_ap, src_row, nrows, w_t, b_t, dst_ap, dst_row):
                nch = nrows // 128
                xT = spool.tile([128, nch, 128], BF16, tag=f"pp_x{nrows}")
                nc.sync.dma_start_transpose(
                    out=xT[:], in_=src_ap[src_row:src_row + nrows, :])
                y = spool.tile([128, nch, 128], BF16, tag=f"pp_y{nrows}")
                for sub in range(nch // 4):
                    pS = ppool.tile([128, 4, 128], F32, space="PSUM",
                                    tag="pp_s")
                    for c in range(4):
                        nc.tensor.matmul(pS[:, c, :],
                                         lhsT=xT[:, sub * 4 + c, :],
                                         rhs=w_t[:], start=True, stop=True,
                                         skip_group_check=True)
                    nc.vector.tensor_tensor(
                        y[:, sub * 4:(sub + 1) * 4, :], pS[:],
                        b_t[:].unsqueeze(1).to_broadcast([128, 4, 128]),
                        op=mybir.AluOpType.add)
                nc.scalar.dma_start(
                    out=dst_ap[dst_row:dst_row + nrows, :]
                        .rearrange("(c p) f -> p c f", p=128),
                    in_=y[:])

            for row in range(0, SPLIT, 2048):
                project(nodes_pad, row, 2048, ws_t, wsb_t, tab_lo, row)
            row = SPLIT
            while row < NP_PAD:
                nrows = min(2048, NP_PAD - row)
                project(nodes_pad, row, nrows, ws_t, wsb_t, tab_hi,
                        row - SPLIT)
                row += nrows
            row = 0
            while row < NL_PAD:
                nrows = min(2048, NL_PAD - row)
                project(nodes_loc, row, nrows, wr_t, wrb_t, tab_r, row)
                row += nrows

        tc.strict_bb_all_engine_barrier()

        # ---------------- Phase 2: edge stream ----------------
        tabs = {"lo": tab_lo, "hi": tab_hi}
        with tc.tile_pool(name="mc", bufs=1) as cpool2, \
             tc.tile_pool(name="stage", bufs=2) as stpool, \
             tc.tile_pool(name="rwinp", bufs=3) as rwpool, \
             tc.tile_pool(name="work", bufs=2) as wpool, \
             tc.tile_pool(name="mid", bufs=1) as mpool, \
             tc.tile_pool(name="psR", bufs=3, space="PSUM") as psR, \
             tc.tile_pool(name="psA", bufs=2, space="PSUM") as psA:
            iota_t = cpool2.tile([128, 128], BF16)
            iotac_t = cpool2.tile([128, 1], F32)
            attn_t = cpool2.tile([128, 128], BF16)
            ident_t = cpool2.tile([128, 128], BF16)
            nc.sync.dma_start(out=iota_t[:], in_=iota_in[:])
            nc.sync.dma_start(out=iotac_t[:], in_=iotac_in[:])
            nc.sync.dma_start(out=attn_t[:], in_=attn_in[:])
            make_identity(nc, ident_t[:])

            acc_ps = None
            rwin_t = None
            qn = [0]

            for b in range(nblk):
                g0 = b * BLK
                rc = min(BLK, n_real - g0)
                if rc <= 0:
                    break
                sidx_t = stpool.tile([128, BLK * CHUNK // 16], I16, tag="sidx")
                rloc_t = stpool.tile([128, BLK], F32, tag="rloc")
                repf_t = stpool.tile([128, BLK * CHUNK], BF16, tag="repf")
                nc.scalar.dma_start(out=sidx_t[:], in_=sidx_in[b])
                nc.scalar.dma_start(out=rloc_t[:], in_=rloc_in[b])
                nc.sync.dma_start(
                    out=repf_t[:],
                    in_=rlocf_in[b].to_broadcast([128, BLK * CHUNK]))
                s_t = stpool.tile([128, BLK, 128], BF16, tag="s_t")
                segs = []
                for cc in range(rc):
                    t = meta[g0 + cc][1]
                    if segs and segs[-1][0] == t and segs[-1][2] < GRP:
                        segs[-1][2] += 1
                    else:
                        segs.append([t, cc, 1])
                for t, cs, nchk in segs:
                    nc.gpsimd.dma_gather(
                        out_ap=s_t[:, cs:cs + nchk, :], in_ap=tabs[t][:],
                        idxs_ap=sidx_t[:, cs * 8:(cs + nchk) * 8],
                        num_idxs=nchk * CHUNK, num_idxs_reg=nchk * CHUNK,
                        elem_size=F, queue_num=qn[0])
                    qn[0] = (qn[0] + 1) % NQ

                # one-hots: ohT (window nodes on partitions, for the r-select
                # matmul) from the partition-broadcast rloc; oh (edges on
                # partitions, for the scatter) per chunk via tensor_scalar
                ohT_t = wpool.tile([128, BLK * CHUNK], BF16, tag="ohT")
                nc.vector.tensor_scalar(
                    ohT_t[:, :rc * CHUNK], repf_t[:, :rc * CHUNK],
                    iotac_t[:], None, op0=mybir.AluOpType.is_equal)
                oh_t = wpool.tile([128, BLK, 128], BF16, tag="oh")
                for cc in range(rc):
                    nc.vector.tensor_scalar(
                        oh_t[:, cc, :], iota_t[:], rloc_t[:, cc:cc + 1], None,
                        op0=mybir.AluOpType.is_equal)

                # per-unit: PSUM x = s_proj (identity mm) + r_proj (one-hot
                # mm against the window tile); x copied out on the scalar
                # engine so DVE ops downstream all run on bf16 SBUF tiles
                x_t = wpool.tile([128, BLK, 128], BF16, tag="x")
                nunit = (rc + UNIT - 1) // UNIT
                for u in range(nunit):
                    c0 = u * UNIT
                    cn = min(UNIT, rc - c0)
                    r_ps = psR.tile([128, UNIT, 128], F32, space="PSUM",
                                    tag="r_ps")
                    nc.tensor.matmul(r_ps[:, :cn, :], lhsT=ident_t[:],
                                     rhs=s_t[:, c0:c0 + cn, :], start=True,
                                     stop=False, skip_group_check=True)
                    for j in range(cn):
                        g = g0 + c0 + j
                        w, _, first, _ = meta[g]
                        if first:
                            rwin_t = rwpool.tile([128, 128], BF16, tag="rwin")
                            nc.sync.dma_start(
                                out=rwin_t[:],
                                in_=tab_r[w * 128:(w + 1) * 128, :])
                        nc.tensor.matmul(
                            r_ps[:, j, :],
                            lhsT=ohT_t[:, (c0 + j) * CHUNK:(c0 + j + 1) * CHUNK],
                            rhs=rwin_t[:], start=False, stop=(j == cn - 1),
                            skip_group_check=True)
                    nc.scalar.activation(x_t[:, c0:c0 + cn, :],
                                         r_ps[:, :cn, :],
                                         mybir.ActivationFunctionType.Copy)

                # mish(x) = x * (1 - 2/((1+e^x)^2+1)): exp+square on the
                # scalar engine (both in the exp_and_others table set), the
                # reciprocal chain on DVE in f32, everything else bf16 2x
                u_t = mpool.tile([128, BLK, 128], BF16, tag="u")
                nc.scalar.activation(u_t[:, :rc, :], x_t[:, :rc, :],
                                     mybir.ActivationFunctionType.Exp)
                q_t = mpool.tile([128, BLK, 128], F32, tag="q")
                nc.scalar.activation(q_t[:, :rc, :], u_t[:, :rc, :],
                                     mybir.ActivationFunctionType.Square,
                                     bias=1.0)
                nc.vector.tensor_scalar_add(q_t[:, :rc, :], q_t[:, :rc, :],
                                            1.0)
                rcp_t = mpool.tile([128, BLK, 128], F32, tag="rcp")
                nc.vector.reciprocal_approx_fast(rcp_t[:, :rc, :],
                                                 q_t[:, :rc, :])
                rr_t = mpool.tile([128, BLK, 128], BF16, tag="rr")
                nc.vector.tensor_scalar(rr_t[:, :rc, :], rcp_t[:, :rc, :],
                                        -2.0, 1.0,
                                        op0=mybir.AluOpType.mult,
                                        op1=mybir.AluOpType.add)
                h_t = mpool.tile([128, BLK, 128], BF16, tag="h")
                nc.vector.tensor_tensor(h_t[:, :rc, :], x_t[:, :rc, :],
                                        rr_t[:, :rc, :],
                                        op=mybir.AluOpType.mult)

                # per-head logits: ha = mish * attn, then pairwise-add tree
                nc.vector.tensor_tensor(
                    h_t[:, :rc, :], h_t[:, :rc, :],
                    attn_t[:].unsqueeze(1).to_broadcast([128, rc, 128]),
                    op=mybir.AluOpType.mult)
                hv = h_t[:, :rc, :].rearrange("p c (h d) -> p c h d", d=D)
                t8 = mpool.tile([128, BLK, H, 8], BF16, tag="t8")
                nc.vector.tensor_tensor(t8[:, :rc], hv[:, :, :, 0:8],
                                        hv[:, :, :, 8:16],
                                        op=mybir.AluOpType.add)
                t4 = mpool.tile([128, BLK, H, 4], BF16, tag="t4")
                nc.vector.tensor_tensor(t4[:, :rc], t8[:, :rc, :, 0:4],
                                        t8[:, :rc, :, 4:8],
                                        op=mybir.AluOpType.add)
                t2 = mpool.tile([128, BLK, H, 2], BF16, tag="t2")
                nc.vector.tensor_tensor(t2[:, :rc], t4[:, :rc, :, 0:2],
                                        t4[:, :rc, :, 2:4],
                                        op=mybir.AluOpType.add)
                lgt_t = mpool.tile([128, BLK, H], BF16, tag="lgt")
                nc.vector.tensor_tensor(lgt_t[:, :rc, :].unsqueeze(3),
                                        t2[:, :rc, :, 0:1], t2[:, :rc, :, 1:2],
                                        op=mybir.AluOpType.add)

                # softmax weight, expanded head->features on the scalar
                # engine so msg runs in 2x DVE mode
                pT128_t = wpool.tile([128, BLK, 128], BF16, tag="pT128")
                nc.scalar.activation(
                    pT128_t[:, :rc, :].rearrange("p c (h d) -> p c h d", d=D),
                    lgt_t[:, :rc, :].unsqueeze(3).to_broadcast([128, rc, H, D]),
                    mybir.ActivationFunctionType.Exp, bias=float(attn_bias))
                rhs_t = wpool.tile([128, BLK, 136], BF16, tag="rhs")
                nc.scalar.activation(rhs_t[:, :rc, 128:136], lgt_t[:, :rc, :],
                                     mybir.ActivationFunctionType.Exp,
                                     bias=float(attn_bias))
                nc.vector.tensor_tensor(rhs_t[:, :rc, :128], s_t[:, :rc, :],
                                        pT128_t[:, :rc, :],
                                        op=mybir.AluOpType.mult)

                for cc in range(rc):
                    g = g0 + cc
                    w, _, first, last = meta[g]
                    if first:
                        acc_ps = psA.tile([128, 136], F32, space="PSUM",
                                          tag="agg")
                    nc.tensor.matmul(acc_ps[:],
                                     lhsT=oh_t[:, cc, :],
                                     rhs=rhs_t[:, cc, :], start=first,
                                     stop=last, skip_group_check=True)
                    if last:
                        # normalize straight out of PSUM and store the window
                        denw = mpool.tile([128, H], F32, tag="denw")
                        nc.vector.tensor_scalar_add(denw[:],
                                                    acc_ps[:, 128:136], 1e-30)
                        rcpw = mpool.tile([128, H], F32, tag="rcpw")
                        nc.vector.reciprocal_approx_fast(rcpw[:], denw[:])
                        outw = mpool.tile([128, 128], F32, tag="outw")
                        nc.vector.tensor_tensor(
                            outw[:].rearrange("p (h d) -> p h d", d=D),
                            acc_ps[:, :128].rearrange("p (h d) -> p h d", d=D),
                            rcpw[:].unsqueeze(2).to_broadcast([128, H, D]),
                            op=mybir.AluOpType.mult)
                        nc.sync.dma_start(
                            out=out_d[w * 128:(w + 1) * 128, :], in_=outw[:])

    nc.compile()
    return nc


def _prep_core(senders, receivers, core, wstruct, nblk):
    """Build sidx/rloc/rlocf arrays for one core given the chunk structure."""
    e_pad = nblk * BLK * CHUNK
    mask = (receivers // NPC) == core
    s = senders[mask].astype(np.int64)
    r = (receivers[mask] - core * NPC).astype(np.int64)
    win = r // WIN
    half = (s >= SPLIT).astype(np.int64)
    order = np.lexsort((half, win))
    s, r, win, half = s[order], r[order], win[order], half[order]

    sidx_val = np.zeros(e_pad, np.int64)
    rloc_val = np.full(e_pad, 999.0, np.float32)

    base = 0
    key = win * 2 + half
    for w, (lw, hw) in enumerate(wstruct):
        for hf, cap_ch in ((0, lw), (1, hw)):
            gmask = key == (w * 2 + hf)
            n = int(gmask.sum())
            cap = cap_ch * CHUNK
            assert n <= cap, f"window {w} half {hf}: {n} > {cap}"
            if n:
                sg = s[gmask]
                sidx_val[base:base + n] = sg - (SPLIT if hf else 0)
                rloc_val[base:base + n] = (r[gmask] - w * WIN).astype(
                    np.float32)
            base += cap

    def wrap16(vals):
        v = vals.reshape(nblk, BLK * CHUNK // 16, 16).astype(np.int16)
        v = np.transpose(v, (0, 2, 1))          # [nblk, 16, 256]
        return np.tile(v, (1, 8, 1)).copy()     # [nblk, 128, 256]

    sidx = wrap16(sidx_val)
    rloc = rloc_val.reshape(nblk, BLK, CHUNK).transpose(0, 2, 1).copy()
    rlocf = rloc_val.reshape(nblk, 1, BLK * CHUNK).astype(BF).copy()
    return sidx, rloc, rlocf


def kernel(nodes, senders, receivers, Ws_k, Ws_b, Wr_k, Wr_b, attn_k, attn_b):
    nodes = np.asarray(nodes, np.float32)
    senders = np.asarray(senders, np.int32)
    receivers = np.asarray(receivers, np.int32)
    assert nodes.shape == (N_NODES, F) and senders.shape == (N_EDGES,)

    # per-window chunk structure: max count across the 8 cores per window
    core_of = receivers // NPC
    r_loc = receivers - core_of * NPC
    win = r_loc // WIN
    half = (senders >= SPLIT).astype(np.int64)
    key = (core_of.astype(np.int64) * NWIN + win) * 2 + half
    counts = np.bincount(key, minlength=NCORE * NWIN * 2).reshape(
        NCORE, NWIN, 2)
    lo_w = np.ceil(counts[:, :, 0].max(axis=0) / CHUNK).astype(int)
    hi_w = np.ceil(counts[:, :, 1].max(axis=0) / CHUNK).astype(int)
    lo_w = np.maximum(lo_w, 1)          # ensure every window has >=1 chunk
    wstruct = tuple((int(a), int(b)) for a, b in zip(lo_w, hi_w))
    n_real = int(lo_w.sum() + hi_w.sum())
    nblk = (n_real + BLK - 1) // BLK

    ck = (wstruct, nblk, float(np.asarray(attn_b).ravel()[0]))
    if ck not in _prog_cache:
        _prog_cache[ck] = _build_program(*ck)
    nc = _prog_cache[ck]

    nodes_bf = nodes.astype(BF)
    nodes_pad = np.zeros((NP_PAD, F), BF)
    nodes_pad[:N_NODES] = nodes_bf
    ws_mat = np.asarray(Ws_k, np.float32).reshape(F, F).astype(BF)
    wr_mat = np.asarray(Wr_k, np.float32).reshape(F, F).astype(BF)
    wsb_rep = np.broadcast_to(
        np.asarray(Ws_b, np.float32).reshape(1, F), (128, F)).astype(BF).copy()
    wrb_rep = np.broadcast_to(
        np.asarray(Wr_b, np.float32).reshape(1, F), (128, F)).astype(BF).copy()
    a_flat = np.tile(np.asarray(attn_k, np.float32).ravel(), H)
    attn_rep = np.broadcast_to(a_flat, (128, 128)).astype(BF).copy()
    iota = np.broadcast_to(np.arange(128, dtype=np.float32),
                           (128, 128)).astype(BF).copy()
    iotac = np.arange(128, dtype=np.float32)[:, None].copy()

    in_maps = []
    for c in range(NCORE):
        sidx, rloc, rlocf = _prep_core(senders, receivers, c, wstruct, nblk)
        nodes_loc = np.zeros((NL_PAD, F), BF)
        nodes_loc[:NPC] = nodes_bf[c * NPC:(c + 1) * NPC]
        in_maps.append({
            "nodes_pad": nodes_pad, "nodes_loc": nodes_loc,
            "ws_mat": ws_mat, "wr_mat": wr_mat,
            "wsb_rep": wsb_rep, "wrb_rep": wrb_rep,
            "iota": iota, "iotac": iotac, "attn_rep": attn_rep,
            "sidx": sidx, "rloc": rloc, "rlocf": rlocf,
        })

    trace = bool(int(os.environ.get("GAT_TRACE", "0")))
    res = bass_utils.run_bass_kernel_spmd(nc, in_maps,
                                          core_ids=list(range(NCORE)),
                                          trace=trace)
    if trace:
        kernel.last_profile = res
    out = np.empty((N_NODES, F), np.float32)
    for c in range(NCORE):
        out[c * NPC:(c + 1) * NPC] = np.asarray(res.results[c]["out_d"])[:NPC]
    return out


# revision 23
# speedup vs baseline: 2.5997x; 1.0874x over previous
"""GATv2 message passing on 8 Trainium2 NeuronCores (Bass/Tile).

Strategy (edge-parallel by receiver ownership, bf16 compute):
  - Nodes are split into 8 contiguous ranges of 6250; core c owns range c and
    all edges whose receiver falls in it (no cross-core reduction needed).
  - Phase 1: each core projects the full node table with Ws (+Ws_b) and its
    local slice with Wr (+Wr_b) into bf16 DRAM tables.  2048-row groups are
    loaded pre-transposed through the HWDGE xbar (dma_start_transpose), the
    PE runs the projection matmuls, the bias add rides the PSUM->SBUF copy
    on the vector engine, and the table write-back goes out on the scalar
    engine's HWDGE queue so loads and stores issue in parallel.
  - Phase 2: stream the edge shard sorted by (receiver window, sender>=32768),
    with a per-window chunk budget set by the max count across the 8 cores
    (the SPMD program is data-dependent; the compile is cached per graph
    structure).  Sender rows come from a bf16 dma_gather (SWDGE, 4 descriptor
    queues round-robin so desc-gen overlaps DMA drain).  Receiver rows are
    NOT gathered: a window holds only 128 receiver nodes, so each chunk
    selects its rows from the window-resident r-projection tile with a
    one-hot matmul; the sender rows are accumulated into the same PSUM via
    an identity matmul, so x = s_proj + r_proj comes out of PSUM with one
    scalar-engine copy.  mish is exp+square on the scalar engine (both live
    in the exp_and_others activation table, so no table reloads) plus an
    approximate-reciprocal chain on DVE; logits finish with a pairwise-add
    reduce tree (2x DVE mode) and the softmax weight is expanded
    head->features on the scalar engine so the msg multiply also runs in 2x
    mode.  Softmax skips the max pass (logits are O(1) so exp is safe).
    The scatter-add and the softmax denominator share one 136-column matmul
    per chunk into per-window PSUM accumulators, and each window is
    normalized straight out of PSUM and stored.
  - out[n] = segsum(exp(logit)*msg) / segsum(exp(logit)), computed on-device;
    host only reassembles the [50000,128] output from the 8 slices.
"""

import os
import sys

for _p in ("/opt/trn_rl_repo", "/root/.axon_site/_ro/trn_rl_repo"):
    if os.path.isdir(_p) and _p not in sys.path:
        sys.path.insert(0, _p)

import numpy as np
import ml_dtypes

import concourse.bass as bass
import concourse.bacc as bacc
import concourse.tile as tile
from concourse import mybir
from concourse import bass_utils
from concourse.masks import make_identity

F32 = mybir.dt.float32
BF16 = mybir.dt.bfloat16
I16 = mybir.dt.int16
BF = ml_dtypes.bfloat16

N_NODES = 50000
N_EDGES = 800000
F = 128            # feature dim
H = 8              # heads
D = 16             # head dim
NCORE = 8
NPC = N_NODES // NCORE          # 6250 nodes per core
WIN = 128                       # nodes per scatter window
NWIN = (NPC + WIN - 1) // WIN   # 49 windows per core
SPLIT = 32768                   # int16 gather-index limit -> lo/hi tables
NP_PAD = 50176                  # nodes padded to 98*512 for precompute
NL_PAD = 6656                   # local nodes padded to 13*512
HI_ROWS = NP_PAD - SPLIT        # 17408
CHUNK = 128                     # edges per matmul chunk
UNIT = 4                        # chunks per r-select PSUM tile
BLK = 32                        # chunks per DMA block (4096 edges)
GRP = 8                         # max chunks per dma_gather call
NQ = 4                          # SWDGE descriptor queues (round-robin)

_prog_cache = {}


def _chunk_meta(wstruct):
    """Per-chunk (window, table, first, last) from per-window chunk counts."""
    meta = []
    for w, (lw, hw) in enumerate(wstruct):
        cw = lw + hw
        for pos in range(cw):
            meta.append((w, "lo" if pos < lw else "hi",
                         pos == 0, pos == cw - 1))
    return meta


def _build_program(wstruct, nblk, attn_bias):
    """Build the SPMD Bass program for the per-window chunk structure."""
    meta = _chunk_meta(wstruct)
    n_real = len(meta)
    assert nblk * BLK >= n_real

    nc = bacc.Bacc("TRN2", target_bir_lowering=False, debug=False,
                   enable_asserts=False, num_devices=NCORE,
                   num_swdge_queues=NQ)

    def dram_in(name, shape, dt=BF16):
        return nc.dram_tensor(name, list(shape), dt, kind="ExternalInput").ap()

    nodes_padT = dram_in("nodes_padT", (128, NP_PAD))
    nodes_locT = dram_in("nodes_locT", (128, NL_PAD))
    ws_mat = dram_in("ws_mat", (F, F))
    wr_mat = dram_in("wr_mat", (F, F))
    wsb_rep = dram_in("wsb_rep", (128, F))
    wrb_rep = dram_in("wrb_rep", (128, F))
    iotac_in = dram_in("iotac", (128, 1), F32)
    attn_in = dram_in("attn_rep", (128, 128))
    sidx_in = dram_in("sidx", (nblk, 128, BLK * CHUNK // 16), I16)
    ohh_in = dram_in("ohh", (nblk, 128, BLK * CHUNK))
    rlocf_in = dram_in("rlocf", (nblk, 1, BLK * CHUNK))
    out_d = nc.dram_tensor("out_d", [NWIN * WIN, F], F32, kind="ExternalOutput").ap()

    tab_lo = nc.dram_tensor("tab_lo", [SPLIT, F], BF16, kind="Internal").ap()
    tab_hi = nc.dram_tensor("tab_hi", [HI_ROWS, F], BF16, kind="Internal").ap()
    tab_r = nc.dram_tensor("tab_r", [NL_PAD, F], BF16, kind="Internal").ap()

    with tile.TileContext(nc) as tc:
        # ---------------- Phase 1: projection precompute ----------------
        with tc.tile_pool(name="pp_const", bufs=1) as cpool, \
             tc.tile_pool(name="pp_sbuf", bufs=3) as spool, \
             tc.tile_pool(name="pp_psum", bufs=2, space="PSUM") as ppool:
            ws_t = cpool.tile([F, F], BF16)
            wr_t = cpool.tile([F, F], BF16)
            wsb_t = cpool.tile([128, F], BF16)
            wrb_t = cpool.tile([128, F], BF16)
            nc.sync.dma_start(out=ws_t[:], in_=ws_mat[:])
            nc.sync.dma_start(out=wr_t[:], in_=wr_mat[:])
            nc.sync.dma_start(out=wsb_t[:], in_=wsb_rep[:])
            nc.sync.dma_start(out=wrb_t[:], in_=wrb_rep[:])

            def project(srcT_ap, src_row, nrows, w_t, b_t, dst_ap, dst_row):
                nch = nrows // 128
                xT = spool.tile([128, nch, 128], BF16, tag=f"pp_x{nrows}")
                nc.sync.dma_start(
                    out=xT[:],
                    in_=srcT_ap[:, src_row:src_row + nrows]
                        .rearrange("p (c k) -> p c k", k=128))
                y = spool.tile([128, nch, 128], BF16, tag=f"pp_y{nrows}")
                for sub in range(nch // 4):
                    pS = ppool.tile([128, 4, 128], F32, space="PSUM",
                                    tag="pp_s")
                    for c in range(4):
                        nc.tensor.matmul(pS[:, c, :],
                                         lhsT=xT[:, sub * 4 + c, :],
                                         rhs=w_t[:], start=True, stop=True,
                                         skip_group_check=True)
                    nc.vector.tensor_tensor(
                        y[:, sub * 4:(sub + 1) * 4, :], pS[:],
                        b_t[:].unsqueeze(1).to_broadcast([128, 4, 128]),
                        op=mybir.AluOpType.add)
                nc.scalar.dma_start(
                    out=dst_ap[dst_row:dst_row + nrows, :]
                        .rearrange("(c p) f -> p c f", p=128),
                    in_=y[:])

            for row in range(0, SPLIT, 2048):
                project(nodes_padT, row, 2048, ws_t, wsb_t, tab_lo, row)
            row = SPLIT
            while row < NP_PAD:
                nrows = min(2048, NP_PAD - row)
                project(nodes_padT, row, nrows, ws_t, wsb_t, tab_hi,
                        row - SPLIT)
                row += nrows
            row = 0
            while row < NL_PAD:
                nrows = min(2048, NL_PAD - row)
                project(nodes_locT, row, nrows, wr_t, wrb_t, tab_r, row)
                row += nrows

        tc.strict_bb_all_engine_barrier()

        # ---------------- Phase 2: edge stream ----------------
        tabs = {"lo": tab_lo, "hi": tab_hi}
        with tc.tile_pool(name="mc", bufs=1) as cpool2, \
             tc.tile_pool(name="gst", bufs=3) as gpool, \
             tc.tile_pool(name="stage", bufs=2) as stpool, \
             tc.tile_pool(name="rwinp", bufs=3) as rwpool, \
             tc.tile_pool(name="work", bufs=2) as wpool, \
             tc.tile_pool(name="mid", bufs=1) as mpool, \
             tc.tile_pool(name="psR", bufs=3, space="PSUM") as psR, \
             tc.tile_pool(name="psA", bufs=2, space="PSUM") as psA:
            iotac_t = cpool2.tile([128, 1], F32)
            attn_t = cpool2.tile([128, 128], BF16)
            ident_t = cpool2.tile([128, 128], BF16)
            nc.sync.dma_start(out=iotac_t[:], in_=iotac_in[:])
            nc.sync.dma_start(out=attn_t[:], in_=attn_in[:])
            make_identity(nc, ident_t[:])

            acc_ps = None
            rwin_t = None
            qn = [0]

            for b in range(nblk):
                g0 = b * BLK
                rc = min(BLK, n_real - g0)
                if rc <= 0:
                    break
                sidx_t = gpool.tile([128, BLK * CHUNK // 16], I16, tag="sidx")
                repf_t = stpool.tile([128, BLK * CHUNK], BF16, tag="repf")
                oh_t = stpool.tile([128, BLK * CHUNK], BF16, tag="oh")
                nc.scalar.dma_start(out=sidx_t[:], in_=sidx_in[b])
                nc.scalar.dma_start(out=oh_t[:], in_=ohh_in[b])
                nc.sync.dma_start(
                    out=repf_t[:],
                    in_=rlocf_in[b].to_broadcast([128, BLK * CHUNK]))
                s_t = gpool.tile([128, BLK, 128], BF16, tag="s_t")
                segs = []
                for cc in range(rc):
                    t = meta[g0 + cc][1]
                    if segs and segs[-1][0] == t and segs[-1][2] < GRP:
                        segs[-1][2] += 1
                    else:
                        segs.append([t, cc, 1])
                for t, cs, nchk in segs:
                    nc.gpsimd.dma_gather(
                        out_ap=s_t[:, cs:cs + nchk, :], in_ap=tabs[t][:],
                        idxs_ap=sidx_t[:, cs * 8:(cs + nchk) * 8],
                        num_idxs=nchk * CHUNK, num_idxs_reg=nchk * CHUNK,
                        elem_size=F, queue_num=qn[0])
                    qn[0] = (qn[0] + 1) % NQ

                # ohT (window nodes on partitions, for the r-select matmul)
                # from the partition-broadcast rloc; oh (edges on partitions,
                # for the scatter) comes host-prepared via DMA
                ohT_t = wpool.tile([128, BLK * CHUNK], BF16, tag="ohT")
                nc.vector.tensor_scalar(
                    ohT_t[:, :rc * CHUNK], repf_t[:, :rc * CHUNK],
                    iotac_t[:], None, op0=mybir.AluOpType.is_equal)

                # per-unit: PSUM x = s_proj (identity mm) + r_proj (one-hot
                # mm against the window tile); x copied out on the scalar
                # engine so DVE ops downstream all run on bf16 SBUF tiles
                x_t = wpool.tile([128, BLK, 128], BF16, tag="x")
                nunit = (rc + UNIT - 1) // UNIT
                for u in range(nunit):
                    c0 = u * UNIT
                    cn = min(UNIT, rc - c0)
                    r_ps = psR.tile([128, UNIT, 128], F32, space="PSUM",
                                    tag="r_ps")
                    nc.tensor.matmul(r_ps[:, :cn, :], lhsT=ident_t[:],
                                     rhs=s_t[:, c0:c0 + cn, :], start=True,
                                     stop=False, skip_group_check=True)
                    for j in range(cn):
                        g = g0 + c0 + j
                        w, _, first, _ = meta[g]
                        if first:
                            rwin_t = rwpool.tile([128, 128], BF16, tag="rwin")
                            nc.sync.dma_start(
                                out=rwin_t[:],
                                in_=tab_r[w * 128:(w + 1) * 128, :])
                        nc.tensor.matmul(
                            r_ps[:, j, :],
                            lhsT=ohT_t[:, (c0 + j) * CHUNK:(c0 + j + 1) * CHUNK],
                            rhs=rwin_t[:], start=False, stop=(j == cn - 1),
                            skip_group_check=True)
                    nc.scalar.activation(x_t[:, c0:c0 + cn, :],
                                         r_ps[:, :cn, :],
                                         mybir.ActivationFunctionType.Copy)

                # mish(x) = x * (1 - 2/((1+e^x)^2+1)): exp+square on the
                # scalar engine (both in the exp_and_others table set), the
                # reciprocal chain on DVE in f32, everything else bf16 2x
                u_t = mpool.tile([128, BLK, 128], BF16, tag="u")
                nc.scalar.activation(u_t[:, :rc, :], x_t[:, :rc, :],
                                     mybir.ActivationFunctionType.Exp)
                q_t = mpool.tile([128, BLK, 128], F32, tag="q")
                nc.scalar.activation(q_t[:, :rc, :], u_t[:, :rc, :],
                                     mybir.ActivationFunctionType.Square,
                                     bias=1.0)
                nc.vector.tensor_scalar_add(q_t[:, :rc, :], q_t[:, :rc, :],
                                            1.0)
                rcp_t = mpool.tile([128, BLK, 128], F32, tag="rcp")
                nc.vector.reciprocal_approx_fast(rcp_t[:, :rc, :],
                                                 q_t[:, :rc, :])
                rr_t = mpool.tile([128, BLK, 128], BF16, tag="rr")
                nc.vector.tensor_scalar(rr_t[:, :rc, :], rcp_t[:, :rc, :],
                                        -2.0, 1.0,
                                        op0=mybir.AluOpType.mult,
                                        op1=mybir.AluOpType.add)
                h_t = mpool.tile([128, BLK, 128], BF16, tag="h")
                nc.vector.tensor_tensor(h_t[:, :rc, :], x_t[:, :rc, :],
                                        rr_t[:, :rc, :],
                                        op=mybir.AluOpType.mult)

                # per-head logits: ha = mish * attn, then pairwise-add tree
                nc.vector.tensor_tensor(
                    h_t[:, :rc, :], h_t[:, :rc, :],
                    attn_t[:].unsqueeze(1).to_broadcast([128, rc, 128]),
                    op=mybir.AluOpType.mult)
                hv = h_t[:, :rc, :].rearrange("p c (h d) -> p c h d", d=D)
                t8 = mpool.tile([128, BLK, H, 8], BF16, tag="t8")
                nc.vector.tensor_tensor(t8[:, :rc], hv[:, :, :, 0:8],
                                        hv[:, :, :, 8:16],
                                        op=mybir.AluOpType.add)
                t4 = mpool.tile([128, BLK, H, 4], BF16, tag="t4")
                nc.vector.tensor_tensor(t4[:, :rc], t8[:, :rc, :, 0:4],
                                        t8[:, :rc, :, 4:8],
                                        op=mybir.AluOpType.add)
                t2 = mpool.tile([128, BLK, H, 2], BF16, tag="t2")
                nc.vector.tensor_tensor(t2[:, :rc], t4[:, :rc, :, 0:2],
                                        t4[:, :rc, :, 2:4],
                                        op=mybir.AluOpType.add)
                lgt_t = mpool.tile([128, BLK, H], BF16, tag="lgt")
                nc.vector.tensor_tensor(lgt_t[:, :rc, :].unsqueeze(3),
                                        t2[:, :rc, :, 0:1], t2[:, :rc, :, 1:2],
                                        op=mybir.AluOpType.add)

                # softmax weight, expanded head->features on the scalar
                # engine so msg runs in 2x DVE mode
                pT128_t = wpool.tile([128, BLK, 128], BF16, tag="pT128")
                nc.scalar.activation(
                    pT128_t[:, :rc, :].rearrange("p c (h d) -> p c h d", d=D),
                    lgt_t[:, :rc, :].unsqueeze(3).to_broadcast([128, rc, H, D]),
                    mybir.ActivationFunctionType.Exp, bias=float(attn_bias))
                rhs_t = wpool.tile([128, BLK, 136], BF16, tag="rhs")
                nc.scalar.activation(rhs_t[:, :rc, 128:136], lgt_t[:, :rc, :],
                                     mybir.ActivationFunctionType.Exp,
                                     bias=float(attn_bias))
                nc.vector.tensor_tensor(rhs_t[:, :rc, :128], s_t[:, :rc, :],
                                        pT128_t[:, :rc, :],
                                        op=mybir.AluOpType.mult)

                for cc in range(rc):
                    g = g0 + cc
                    w, _, first, last = meta[g]
                    if first:
                        acc_ps = psA.tile([128, 136], F32, space="PSUM",
                                          tag="agg")
                    nc.tensor.matmul(acc_ps[:],
                                     lhsT=oh_t[:, cc * CHUNK:(cc + 1) * CHUNK],
                                     rhs=rhs_t[:, cc, :], start=first,
                                     stop=last, skip_group_check=True)
                    if last:
                        # normalize straight out of PSUM and store the window
                        denw = mpool.tile([128, H], F32, tag="denw")
                        nc.vector.tensor_scalar_add(denw[:],
                                                    acc_ps[:, 128:136], 1e-30)
                        rcpw = mpool.tile([128, H], F32, tag="rcpw")
                        nc.vector.reciprocal_approx_fast(rcpw[:], denw[:])
                        outw = mpool.tile([128, 128], F32, tag="outw")
                        nc.vector.tensor_tensor(
                            outw[:].rearrange("p (h d) -> p h d", d=D),
                            acc_ps[:, :128].rearrange("p (h d) -> p h d", d=D),
                            rcpw[:].unsqueeze(2).to_broadcast([128, H, D]),
                            op=mybir.AluOpType.mult)
                        nc.sync.dma_start(
                            out=out_d[w * 128:(w + 1) * 128, :], in_=outw[:])

    nc.compile()
    return nc


def _prep_core(senders, receivers, core, wstruct, nblk):
    """Build sidx/rloc/rlocf arrays for one core given the chunk structure."""
    e_pad = nblk * BLK * CHUNK
    mask = (receivers // NPC) == core
    s = senders[mask].astype(np.int64)
    r = (receivers[mask] - core * NPC).astype(np.int64)
    win = r // WIN
    half = (s >= SPLIT).astype(np.int64)
    order = np.lexsort((half, win))
    s, r, win, half = s[order], r[order], win[order], half[order]

    sidx_val = np.zeros(e_pad, np.int64)
    rloc_val = np.full(e_pad, 999.0, np.float32)

    base = 0
    key = win * 2 + half
    for w, (lw, hw) in enumerate(wstruct):
        for hf, cap_ch in ((0, lw), (1, hw)):
            gmask = key == (w * 2 + hf)
            n = int(gmask.sum())
            cap = cap_ch * CHUNK
            assert n <= cap, f"window {w} half {hf}: {n} > {cap}"
            if n:
                sg = s[gmask]
                sidx_val[base:base + n] = sg - (SPLIT if hf else 0)
                rloc_val[base:base + n] = (r[gmask] - w * WIN).astype(
                    np.float32)
            base += cap

    def wrap16(vals):
        v = vals.reshape(nblk, BLK * CHUNK // 16, 16).astype(np.int16)
        v = np.transpose(v, (0, 2, 1))          # [nblk, 16, 256]
        return np.tile(v, (1, 8, 1)).copy()     # [nblk, 128, 256]

    sidx = wrap16(sidx_val)
    rv = rloc_val.reshape(nblk, BLK, CHUNK)     # [b, c, p]
    oh = (rv[:, :, :, None] == np.arange(128, dtype=np.float32))
    ohh = np.ascontiguousarray(
        oh.transpose(0, 2, 1, 3)).reshape(nblk, CHUNK, BLK * 128)
    ohh = ohh.astype(np.float32).astype(BF)
    rlocf = rloc_val.reshape(nblk, 1, BLK * CHUNK).astype(BF).copy()
    return sidx, ohh, rlocf


def kernel(nodes, senders, receivers, Ws_k, Ws_b, Wr_k, Wr_b, attn_k, attn_b):
    nodes = np.asarray(nodes, np.float32)
    senders = np.asarray(senders, np.int32)
    receivers = np.asarray(receivers, np.int32)
    assert nodes.shape == (N_NODES, F) and senders.shape == (N_EDGES,)

    # per-window chunk structure: max count across the 8 cores per window
    core_of = receivers // NPC
    r_loc = receivers - core_of * NPC
    win = r_loc // WIN
    half = (senders >= SPLIT).astype(np.int64)
    key = (core_of.astype(np.int64) * NWIN + win) * 2 + half
    counts = np.bincount(key, minlength=NCORE * NWIN * 2).reshape(
        NCORE, NWIN, 2)
    lo_w = np.ceil(counts[:, :, 0].max(axis=0) / CHUNK).astype(int)
    hi_w = np.ceil(counts[:, :, 1].max(axis=0) / CHUNK).astype(int)
    lo_w = np.maximum(lo_w, 1)          # ensure every window has >=1 chunk
    wstruct = tuple((int(a), int(b)) for a, b in zip(lo_w, hi_w))
    n_real = int(lo_w.sum() + hi_w.sum())
    nblk = (n_real + BLK - 1) // BLK

    ck = (wstruct, nblk, float(np.asarray(attn_b).ravel()[0]))
    if ck not in _prog_cache:
        _prog_cache[ck] = _build_program(*ck)
    nc = _prog_cache[ck]

    nodes_bf = nodes.astype(BF)
    nodes_padT = np.zeros((128, NP_PAD), BF)
    nodes_padT[:, :N_NODES] = nodes_bf.T
    ws_mat = np.asarray(Ws_k, np.float32).reshape(F, F).astype(BF)
    wr_mat = np.asarray(Wr_k, np.float32).reshape(F, F).astype(BF)
    wsb_rep = np.broadcast_to(
        np.asarray(Ws_b, np.float32).reshape(1, F), (128, F)).astype(BF).copy()
    wrb_rep = np.broadcast_to(
        np.asarray(Wr_b, np.float32).reshape(1, F), (128, F)).astype(BF).copy()
    a_flat = np.tile(np.asarray(attn_k, np.float32).ravel(), H)
    attn_rep = np.broadcast_to(a_flat, (128, 128)).astype(BF).copy()
    iotac = np.arange(128, dtype=np.float32)[:, None].copy()

    in_maps = []
    for c in range(NCORE):
        sidx, ohh, rlocf = _prep_core(senders, receivers, c, wstruct, nblk)
        nodes_locT = np.zeros((128, NL_PAD), BF)
        nodes_locT[:, :NPC] = nodes_bf[c * NPC:(c + 1) * NPC].T
        in_maps.append({
            "nodes_padT": nodes_padT, "nodes_locT": nodes_locT,
            "ws_mat": ws_mat, "wr_mat": wr_mat,
            "wsb_rep": wsb_rep, "wrb_rep": wrb_rep,
            "iotac": iotac, "attn_rep": attn_rep,
            "sidx": sidx, "ohh": ohh, "rlocf": rlocf,
        })

    trace = bool(int(os.environ.get("GAT_TRACE", "0")))
    res = bass_utils.run_bass_kernel_spmd(nc, in_maps,
                                          core_ids=list(range(NCORE)),
                                          trace=trace)
    if trace:
        kernel.last_profile = res
    out = np.empty((N_NODES, F), np.float32)
    for c in range(NCORE):
        out[c * NPC:(c + 1) * NPC] = np.asarray(res.results[c]["out_d"])[:NPC]
    return out


# revision 31
# speedup vs baseline: 3.0237x; 1.1631x over previous
"""GATv2 message passing on 8 Trainium2 NeuronCores (Bass/Tile).

Strategy (edge-parallel by receiver ownership, bf16 compute):
  - Nodes are split into 8 contiguous ranges of 6250; core c owns range c and
    all edges whose receiver falls in it (no cross-core reduction needed).
  - Phase 1: each core projects the full node table with Ws (+Ws_b) and its
    local slice with Wr (+Wr_b) into bf16 DRAM tables.  2048-row groups are
    loaded pre-transposed through the HWDGE xbar (dma_start_transpose), the
    PE runs the projection matmuls, the bias add rides the PSUM->SBUF copy
    on the vector engine, and the table write-back goes out on the scalar
    engine's HWDGE queue so loads and stores issue in parallel.
  - Phase 2: stream the edge shard sorted by (receiver window, sender>=32768),
    with a per-window chunk budget set by the max count across the 8 cores
    (the SPMD program is data-dependent; the compile is cached per graph
    structure).  Sender rows come from a bf16 dma_gather (SWDGE, 4 descriptor
    queues round-robin so desc-gen overlaps DMA drain).  Receiver rows are
    NOT gathered: a window holds only 128 receiver nodes, so each chunk
    selects its rows from the window-resident r-projection tile with a
    one-hot matmul; the sender rows are accumulated into the same PSUM via
    an identity matmul, so x = s_proj + r_proj comes out of PSUM with one
    scalar-engine copy.  mish is exp+square on the scalar engine (both live
    in the exp_and_others activation table, so no table reloads) plus an
    approximate-reciprocal chain on DVE; logits finish with a pairwise-add
    reduce tree (2x DVE mode) and the softmax weight is expanded
    head->features on the scalar engine so the msg multiply also runs in 2x
    mode.  Softmax skips the max pass (logits are O(1) so exp is safe).
    The scatter-add and the softmax denominator share one 136-column matmul
    per chunk into per-window PSUM accumulators, and each window is
    normalized straight out of PSUM and stored.
  - out[n] = segsum(exp(logit)*msg) / segsum(exp(logit)), computed on-device;
    host only reassembles the [50000,128] output from the 8 slices.
"""

import os
import sys

for _p in ("/opt/trn_rl_repo", "/root/.axon_site/_ro/trn_rl_repo"):
    if os.path.isdir(_p) and _p not in sys.path:
        sys.path.insert(0, _p)

import numpy as np
import ml_dtypes

import concourse.bass as bass
import concourse.bacc as bacc
import concourse.tile as tile
from concourse import mybir
from concourse import bass_utils
from concourse.masks import make_identity

F32 = mybir.dt.float32
BF16 = mybir.dt.bfloat16
I16 = mybir.dt.int16
BF = ml_dtypes.bfloat16

N_NODES = 50000
N_EDGES = 800000
F = 128            # feature dim
H = 8              # heads
D = 16             # head dim
NCORE = 8
NPC = N_NODES // NCORE          # 6250 nodes per core
WIN = 128                       # nodes per scatter window
NWIN = (NPC + WIN - 1) // WIN   # 49 windows per core
SPLIT = 32768                   # int16 gather-index limit -> lo/hi tables
NP_PAD = 50176                  # nodes padded to 98*512 for precompute
NL_PAD = 6656                   # local nodes padded to 13*512
HI_ROWS = NP_PAD - SPLIT        # 17408
CHUNK = 128                     # edges per matmul chunk
UNIT = 4                        # chunks per r-select PSUM tile
BLK = 32                        # chunks per DMA block (4096 edges)
GRP = 8                         # max chunks per dma_gather call
NQ = 4                          # SWDGE descriptor queues (round-robin)

_prog_cache = {}


def _chunk_meta(wstruct):
    """Per-chunk (window, table, first, last) from per-window chunk counts.
    Windows alternate lo/hi order so adjacent same-table runs merge into
    longer dma_gather calls."""
    meta = []
    for w, (lw, hw) in enumerate(wstruct):
        cw = lw + hw
        halves = ("lo",) * lw + ("hi",) * hw
        if w % 2 == 1:
            halves = halves[::-1]
        for pos in range(cw):
            meta.append((w, halves[pos], pos == 0, pos == cw - 1))
    return meta


def _build_program(wstruct, nblk, attn_bias):
    """Build the SPMD Bass program for the per-window chunk structure."""
    meta = _chunk_meta(wstruct)
    n_real = len(meta)
    assert nblk * BLK >= n_real

    nc = bacc.Bacc("TRN2", target_bir_lowering=False, debug=False,
                   enable_asserts=False, num_devices=NCORE,
                   num_swdge_queues=NQ)

    def dram_in(name, shape, dt=BF16):
        return nc.dram_tensor(name, list(shape), dt, kind="ExternalInput").ap()

    nodes_padT = dram_in("nodes_padT", (128, NP_PAD))
    nodes_locT = dram_in("nodes_locT", (128, NL_PAD))
    ws_mat = dram_in("ws_mat", (F, F))
    wr_mat = dram_in("wr_mat", (F, F))
    wsb_rep = dram_in("wsb_rep", (128, F))
    wrb_rep = dram_in("wrb_rep", (128, F))
    attn_in = dram_in("attn_rep", (128, 128))
    sidx_in = dram_in("sidx", (nblk, 128, BLK * CHUNK // 16), I16)
    ohh_in = dram_in("ohh", (nblk, 128, BLK * CHUNK))
    ohht_in = dram_in("ohht", (nblk, 128, BLK * CHUNK))
    out_d = nc.dram_tensor("out_d", [NWIN * WIN, F], F32, kind="ExternalOutput").ap()

    tab_lo = nc.dram_tensor("tab_lo", [SPLIT, F], BF16, kind="Internal").ap()
    tab_hi = nc.dram_tensor("tab_hi", [HI_ROWS, F], BF16, kind="Internal").ap()
    tab_r = nc.dram_tensor("tab_r", [NL_PAD, F], BF16, kind="Internal").ap()

    with tile.TileContext(nc) as tc:
        # ---------------- Phase 1: projection precompute ----------------
        with tc.tile_pool(name="pp_const", bufs=1) as cpool, \
             tc.tile_pool(name="pp_sbuf", bufs=3) as spool, \
             tc.tile_pool(name="pp_psum", bufs=2, space="PSUM") as ppool:
            ws_t = cpool.tile([F, F], BF16)
            wr_t = cpool.tile([F, F], BF16)
            wsb_t = cpool.tile([128, F], BF16)
            wrb_t = cpool.tile([128, F], BF16)
            nc.sync.dma_start(out=ws_t[:], in_=ws_mat[:])
            nc.sync.dma_start(out=wr_t[:], in_=wr_mat[:])
            nc.sync.dma_start(out=wsb_t[:], in_=wsb_rep[:])
            nc.sync.dma_start(out=wrb_t[:], in_=wrb_rep[:])

            def project(srcT_ap, src_row, nrows, w_t, b_t, dst_ap, dst_row):
                nch = nrows // 128
                xT = spool.tile([128, nch, 128], BF16, tag=f"pp_x{nrows}")
                nc.sync.dma_start(
                    out=xT[:],
                    in_=srcT_ap[:, src_row:src_row + nrows]
                        .rearrange("p (c k) -> p c k", k=128))
                y = spool.tile([128, nch, 128], BF16, tag=f"pp_y{nrows}")
                for sub in range(nch // 4):
                    pS = ppool.tile([128, 4, 128], F32, space="PSUM",
                                    tag="pp_s")
                    for c in range(4):
                        nc.tensor.matmul(pS[:, c, :],
                                         lhsT=xT[:, sub * 4 + c, :],
                                         rhs=w_t[:], start=True, stop=True,
                                         skip_group_check=True)
                    nc.vector.tensor_tensor(
                        y[:, sub * 4:(sub + 1) * 4, :], pS[:],
                        b_t[:].unsqueeze(1).to_broadcast([128, 4, 128]),
                        op=mybir.AluOpType.add)
                nc.scalar.dma_start(
                    out=dst_ap[dst_row:dst_row + nrows, :]
                        .rearrange("(c p) f -> p c f", p=128),
                    in_=y[:])

            for row in range(0, SPLIT, 2048):
                project(nodes_padT, row, 2048, ws_t, wsb_t, tab_lo, row)
            row = SPLIT
            while row < NP_PAD:
                nrows = min(2048, NP_PAD - row)
                project(nodes_padT, row, nrows, ws_t, wsb_t, tab_hi,
                        row - SPLIT)
                row += nrows
            row = 0
            while row < NL_PAD:
                nrows = min(2048, NL_PAD - row)
                project(nodes_locT, row, nrows, wr_t, wrb_t, tab_r, row)
                row += nrows

        tc.strict_bb_all_engine_barrier()

        # ---------------- Phase 2: edge stream ----------------
        tabs = {"lo": tab_lo, "hi": tab_hi}
        with tc.tile_pool(name="mc", bufs=1) as cpool2, \
             tc.tile_pool(name="gst", bufs=3) as gpool, \
             tc.tile_pool(name="stage", bufs=2) as stpool, \
             tc.tile_pool(name="rwinp", bufs=3) as rwpool, \
             tc.tile_pool(name="work", bufs=2) as wpool, \
             tc.tile_pool(name="mid", bufs=1) as mpool, \
             tc.tile_pool(name="psR", bufs=3, space="PSUM") as psR, \
             tc.tile_pool(name="psA", bufs=2, space="PSUM") as psA:
            attn_t = cpool2.tile([128, 128], BF16)
            ident_t = cpool2.tile([128, 128], BF16)
            nc.sync.dma_start(out=attn_t[:], in_=attn_in[:])
            make_identity(nc, ident_t[:])

            acc_ps = None
            rwin_t = None
            qn = [0]

            for b in range(nblk):
                g0 = b * BLK
                rc = min(BLK, n_real - g0)
                if rc <= 0:
                    break
                sidx_t = gpool.tile([128, BLK * CHUNK // 16], I16, tag="sidx")
                oh_t = stpool.tile([128, BLK * CHUNK], BF16, tag="oh")
                ohT_t = stpool.tile([128, BLK * CHUNK], BF16, tag="ohT")
                nc.scalar.dma_start(out=sidx_t[:], in_=sidx_in[b])
                nc.sync.dma_start(out=oh_t[:], in_=ohh_in[b])
                nc.sync.dma_start(out=ohT_t[:], in_=ohht_in[b])
                s_t = gpool.tile([128, BLK, 128], BF16, tag="s_t")
                segs = []
                for cc in range(rc):
                    t = meta[g0 + cc][1]
                    if segs and segs[-1][0] == t and segs[-1][2] < GRP:
                        segs[-1][2] += 1
                    else:
                        segs.append([t, cc, 1])
                for t, cs, nchk in segs:
                    nc.gpsimd.dma_gather(
                        out_ap=s_t[:, cs:cs + nchk, :], in_ap=tabs[t][:],
                        idxs_ap=sidx_t[:, cs * 8:(cs + nchk) * 8],
                        num_idxs=nchk * CHUNK, num_idxs_reg=nchk * CHUNK,
                        elem_size=F, queue_num=qn[0])
                    qn[0] = (qn[0] + 1) % NQ

                # per-unit: PSUM x = s_proj (identity mm) + r_proj (one-hot
                # mm against the window tile); x copied out on the scalar
                # engine so DVE ops downstream all run on bf16 SBUF tiles
                x_t = wpool.tile([128, BLK, 128], BF16, tag="x")
                nunit = (rc + UNIT - 1) // UNIT
                for u in range(nunit):
                    c0 = u * UNIT
                    cn = min(UNIT, rc - c0)
                    r_ps = psR.tile([128, UNIT, 128], F32, space="PSUM",
                                    tag="r_ps")
                    nc.tensor.matmul(r_ps[:, :cn, :], lhsT=ident_t[:],
                                     rhs=s_t[:, c0:c0 + cn, :], start=True,
                                     stop=False, skip_group_check=True)
                    for j in range(cn):
                        g = g0 + c0 + j
                        w, _, first, _ = meta[g]
                        if first:
                            rwin_t = rwpool.tile([128, 128], BF16, tag="rwin")
                            nc.sync.dma_start(
                                out=rwin_t[:],
                                in_=tab_r[w * 128:(w + 1) * 128, :])
                        nc.tensor.matmul(
                            r_ps[:, j, :],
                            lhsT=ohT_t[:, (c0 + j) * CHUNK:(c0 + j + 1) * CHUNK],
                            rhs=rwin_t[:], start=False, stop=(j == cn - 1),
                            skip_group_check=True)
                    nc.scalar.activation(x_t[:, c0:c0 + cn, :],
                                         r_ps[:, :cn, :],
                                         mybir.ActivationFunctionType.Copy)

                # mish(x) = x * (1 - 2/((1+e^x)^2+1)): exp+square on the
                # scalar engine (both in the exp_and_others table set), the
                # reciprocal chain on DVE in f32, everything else bf16 2x
                u_t = mpool.tile([128, BLK, 128], BF16, tag="u")
                nc.scalar.activation(u_t[:, :rc, :], x_t[:, :rc, :],
                                     mybir.ActivationFunctionType.Exp)
                q_t = mpool.tile([128, BLK, 128], F32, tag="q")
                nc.scalar.activation(q_t[:, :rc, :], u_t[:, :rc, :],
                                     mybir.ActivationFunctionType.Square,
                                     bias=1.0)
                nc.vector.tensor_scalar_add(q_t[:, :rc, :], q_t[:, :rc, :],
                                            1.0)
                rcp_t = mpool.tile([128, BLK, 128], F32, tag="rcp")
                nc.vector.reciprocal_approx_fast(rcp_t[:, :rc, :],
                                                 q_t[:, :rc, :])
                rr_t = mpool.tile([128, BLK, 128], BF16, tag="rr")
                nc.vector.tensor_scalar(rr_t[:, :rc, :], rcp_t[:, :rc, :],
                                        -2.0, 1.0,
                                        op0=mybir.AluOpType.mult,
                                        op1=mybir.AluOpType.add)
                h_t = mpool.tile([128, BLK, 128], BF16, tag="h")
                nc.vector.tensor_tensor(h_t[:, :rc, :], x_t[:, :rc, :],
                                        rr_t[:, :rc, :],
                                        op=mybir.AluOpType.mult)

                # per-head logits: ha = mish * attn, then pairwise-add tree
                nc.vector.tensor_tensor(
                    h_t[:, :rc, :], h_t[:, :rc, :],
                    attn_t[:].unsqueeze(1).to_broadcast([128, rc, 128]),
                    op=mybir.AluOpType.mult)
                hv = h_t[:, :rc, :].rearrange("p c (h d) -> p c h d", d=D)
                t8 = mpool.tile([128, BLK, H, 8], BF16, tag="t8")
                nc.vector.tensor_tensor(t8[:, :rc], hv[:, :, :, 0:8],
                                        hv[:, :, :, 8:16],
                                        op=mybir.AluOpType.add)
                t4 = mpool.tile([128, BLK, H, 4], BF16, tag="t4")
                nc.vector.tensor_tensor(t4[:, :rc], t8[:, :rc, :, 0:4],
                                        t8[:, :rc, :, 4:8],
                                        op=mybir.AluOpType.add)
                t2 = mpool.tile([128, BLK, H, 2], BF16, tag="t2")
                nc.vector.tensor_tensor(t2[:, :rc], t4[:, :rc, :, 0:2],
                                        t4[:, :rc, :, 2:4],
                                        op=mybir.AluOpType.add)
                lgt_t = mpool.tile([128, BLK, H], BF16, tag="lgt")
                nc.vector.tensor_tensor(lgt_t[:, :rc, :].unsqueeze(3),
                                        t2[:, :rc, :, 0:1], t2[:, :rc, :, 1:2],
                                        op=mybir.AluOpType.add)

                # softmax weight, expanded head->features on the scalar
                # engine so msg runs in 2x DVE mode
                pT128_t = wpool.tile([128, BLK, 128], BF16, tag="pT128")
                nc.scalar.activation(
                    pT128_t[:, :rc, :].rearrange("p c (h d) -> p c h d", d=D),
                    lgt_t[:, :rc, :].unsqueeze(3).to_broadcast([128, rc, H, D]),
                    mybir.ActivationFunctionType.Exp, bias=float(attn_bias))
                rhs_t = wpool.tile([128, BLK, 136], BF16, tag="rhs")
                nc.scalar.activation(rhs_t[:, :rc, 128:136], lgt_t[:, :rc, :],
                                     mybir.ActivationFunctionType.Exp,
                                     bias=float(attn_bias))
                nc.vector.tensor_tensor(rhs_t[:, :rc, :128], s_t[:, :rc, :],
                                        pT128_t[:, :rc, :],
                                        op=mybir.AluOpType.mult)

                for cc in range(rc):
                    g = g0 + cc
                    w, _, first, last = meta[g]
                    if first:
                        acc_ps = psA.tile([128, 136], F32, space="PSUM",
                                          tag="agg")
                    nc.tensor.matmul(acc_ps[:],
                                     lhsT=oh_t[:, cc * CHUNK:(cc + 1) * CHUNK],
                                     rhs=rhs_t[:, cc, :], start=first,
                                     stop=last, skip_group_check=True)
                    if last:
                        # normalize straight out of PSUM and store the window
                        denw = mpool.tile([128, H], F32, tag="denw")
                        nc.vector.tensor_scalar_add(denw[:],
                                                    acc_ps[:, 128:136], 1e-30)
                        rcpw = mpool.tile([128, H], F32, tag="rcpw")
                        nc.vector.reciprocal_approx_fast(rcpw[:], denw[:])
                        outw = mpool.tile([128, 128], F32, tag="outw")
                        nc.vector.tensor_tensor(
                            outw[:].rearrange("p (h d) -> p h d", d=D),
                            acc_ps[:, :128].rearrange("p (h d) -> p h d", d=D),
                            rcpw[:].unsqueeze(2).to_broadcast([128, H, D]),
                            op=mybir.AluOpType.mult)
                        nc.sync.dma_start(
                            out=out_d[w * 128:(w + 1) * 128, :], in_=outw[:])

    nc.compile()
    return nc


def _prep_core(senders, receivers, core, wstruct, nblk):
    """Build sidx/ohh/ohht arrays for one core given the chunk structure.
    Edges within each (window, half) run are sorted by sender so the gather
    addresses ascend (HBM row/bank locality)."""
    e_pad = nblk * BLK * CHUNK
    mask = (receivers // NPC) == core
    s = senders[mask].astype(np.int64)
    r = (receivers[mask] - core * NPC).astype(np.int64)
    win = r // WIN
    half = (s >= SPLIT).astype(np.int64)
    order = np.lexsort((s, half, win))
    s, r, win, half = s[order], r[order], win[order], half[order]

    sidx_val = np.zeros(e_pad, np.int64)
    rloc_val = np.full(e_pad, 999.0, np.float32)

    base = 0
    key = win * 2 + half
    for w, (lw, hw) in enumerate(wstruct):
        halves = ((0, lw), (1, hw)) if w % 2 == 0 else ((1, hw), (0, lw))
        for hf, cap_ch in halves:
            gmask = key == (w * 2 + hf)
            n = int(gmask.sum())
            cap = cap_ch * CHUNK
            assert n <= cap, f"window {w} half {hf}: {n} > {cap}"
            if n:
                sg = s[gmask]
                sidx_val[base:base + n] = sg - (SPLIT if hf else 0)
                rloc_val[base:base + n] = (r[gmask] - w * WIN).astype(
                    np.float32)
            base += cap

    def wrap16(vals):
        v = vals.reshape(nblk, BLK * CHUNK // 16, 16).astype(np.int16)
        v = np.transpose(v, (0, 2, 1))          # [nblk, 16, 256]
        return np.tile(v, (1, 8, 1)).copy()     # [nblk, 128, 256]

    sidx = wrap16(sidx_val)
    rv = rloc_val.reshape(nblk, BLK, CHUNK)     # [b, c, p]
    iot = np.arange(128, dtype=np.float32)
    oh = (rv[:, :, :, None] == iot)             # [b, c, p(edge), n]
    ohh = np.ascontiguousarray(
        oh.transpose(0, 2, 1, 3)).reshape(nblk, CHUNK, BLK * 128)
    ohh = ohh.astype(np.float32).astype(BF)
    # transposed one-hot: [b, n(node partition), c*128+edge]
    ohht = np.ascontiguousarray(
        oh.transpose(0, 3, 1, 2)).reshape(nblk, 128, BLK * CHUNK)
    ohht = ohht.astype(np.float32).astype(BF)
    return sidx, ohh, ohht


def kernel(nodes, senders, receivers, Ws_k, Ws_b, Wr_k, Wr_b, attn_k, attn_b):
    nodes = np.asarray(nodes, np.float32)
    senders = np.asarray(senders, np.int32)
    receivers = np.asarray(receivers, np.int32)
    assert nodes.shape == (N_NODES, F) and senders.shape == (N_EDGES,)

    # per-window chunk structure: max count across the 8 cores per window
    core_of = receivers // NPC
    r_loc = receivers - core_of * NPC
    win = r_loc // WIN
    half = (senders >= SPLIT).astype(np.int64)
    key = (core_of.astype(np.int64) * NWIN + win) * 2 + half
    counts = np.bincount(key, minlength=NCORE * NWIN * 2).reshape(
        NCORE, NWIN, 2)
    lo_w = np.ceil(counts[:, :, 0].max(axis=0) / CHUNK).astype(int)
    hi_w = np.ceil(counts[:, :, 1].max(axis=0) / CHUNK).astype(int)
    lo_w = np.maximum(lo_w, 1)          # ensure every window has >=1 chunk
    wstruct = tuple((int(a), int(b)) for a, b in zip(lo_w, hi_w))
    n_real = int(lo_w.sum() + hi_w.sum())
    nblk = (n_real + BLK - 1) // BLK

    ck = (wstruct, nblk, float(np.asarray(attn_b).ravel()[0]))
    if ck not in _prog_cache:
        _prog_cache[ck] = _build_program(*ck)
    nc = _prog_cache[ck]

    nodes_bf = nodes.astype(BF)
    nodes_padT = np.zeros((128, NP_PAD), BF)
    nodes_padT[:, :N_NODES] = nodes_bf.T
    ws_mat = np.asarray(Ws_k, np.float32).reshape(F, F).astype(BF)
    wr_mat = np.asarray(Wr_k, np.float32).reshape(F, F).astype(BF)
    wsb_rep = np.broadcast_to(
        np.asarray(Ws_b, np.float32).reshape(1, F), (128, F)).astype(BF).copy()
    wrb_rep = np.broadcast_to(
        np.asarray(Wr_b, np.float32).reshape(1, F), (128, F)).astype(BF).copy()
    a_flat = np.tile(np.asarray(attn_k, np.float32).ravel(), H)
    attn_rep = np.broadcast_to(a_flat, (128, 128)).astype(BF).copy()

    in_maps = []
    for c in range(NCORE):
        sidx, ohh, ohht = _prep_core(senders, receivers, c, wstruct, nblk)
        nodes_locT = np.zeros((128, NL_PAD), BF)
        nodes_locT[:, :NPC] = nodes_bf[c * NPC:(c + 1) * NPC].T
        in_maps.append({
            "nodes_padT": nodes_padT, "nodes_locT": nodes_locT,
            "ws_mat": ws_mat, "wr_mat": wr_mat,
            "wsb_rep": wsb_rep, "wrb_rep": wrb_rep,
            "attn_rep": attn_rep,
            "sidx": sidx, "ohh": ohh, "ohht": ohht,
        })

    trace = bool(int(os.environ.get("GAT_TRACE", "0")))
    res = bass_utils.run_bass_kernel_spmd(nc, in_maps,
                                          core_ids=list(range(NCORE)),
                                          trace=trace)
    if trace:
        kernel.last_profile = res
    out = np.empty((N_NODES, F), np.float32)
    for c in range(NCORE):
        out[c * NPC:(c + 1) * NPC] = np.asarray(res.results[c]["out_d"])[:NPC]
    return out
